# revision 13
# baseline (speedup 1.0000x reference)
"""Two-layer GAT (GATConv x2, 4 heads, head-mean) on 8 TRN2 NeuronCores.

Strategy (graph/data parallel):
  - Nodes sharded contiguously across 8 cores (1250 each). Within each
    shard, nodes are relabeled by degree rank so that the tile structure
    (baked into the single SPMD NEFF) is identical across cores; per-node
    degree is padded to the cross-core max at each rank (dummy edges go to
    a per-tile trash slot and do not affect results).
  - Per layer: sharded projection on TensorE (h via bf16 matmul, attention
    logits asrc/adst via f32 matmul against host-folded W@a vectors), then
    AllGather of the packed node table [h(bf16) | asrc(f32)], then the
    edge phase: per 128-edge tile, indirect-gather source rows, tile-local
    segment softmax via selection-matrix matmuls, head-accumulated PSUM
    aggregation seeded with the bias, relu, contiguous output write.
  - Edge tiles are packed per 128-node destination blocks (block-relative
    slots) so alpha_dst stays SBUF-resident; softmax path is f32, message
    path bf16.
"""
import sys

sys.path.insert(0, "/opt/trn_rl_repo")

import numpy as np
import ml_dtypes

import concourse.bass as bass
import concourse.mybir as mybir
import concourse.tile as tile
from concourse import bacc
from concourse.bass_utils import run_bass_kernel_spmd

P = 128
N_NODES = 10000
N_EDGES = 160000
NC = 8
S = N_NODES // NC          # 1250 nodes per core
NB = (S + P - 1) // P      # 10 dst blocks per core
H = 4
C1, C2 = 256, 128
D_IN = 128
DB1 = H * C1 // 2 + H      # 516 f32 cols: bf16 h | f32 asrc
DB2 = H * C2 // 2 + H      # 260 f32 cols
NEG = 0.2
G8 = 16                    # dlocT replicate batch (tiles per DMA)

_dt = mybir.dt


# ----------------------------------------------------------------- host prep

def _pack_structure(target_deg):
    """Tile packing from the (core-independent) per-rank degree sequence.

    Each tile: <=128 edge slots, all edges of a rank-node kept together,
    tiles never span 128-rank block boundaries. Returns per-tile
    (d0, nd, blk, seg list of (rank, count)).
    """
    tiles = []
    cur, cur_cnt = [], 0
    for r in range(S):
        cnt = int(target_deg[r])
        assert cnt <= P, "degree > 128 unsupported"
        blk_changed = cur and (cur[0][0] // P != r // P)
        if cur_cnt + cnt > P or blk_changed:
            tiles.append(cur)
            cur, cur_cnt = [], 0
        cur.append((r, cnt))
        cur_cnt += cnt
    if cur:
        tiles.append(cur)
    meta = []
    for segs in tiles:
        d0 = segs[0][0]
        nd = segs[-1][0] - d0 + 1
        blk = d0 // P
        assert segs[-1][0] // P == blk and nd <= P
        meta.append((d0, nd, blk, segs))
    return meta


def _host_prepare(x, edge_index, W1, a_src1, a_dst1, b1, W2, a_src2, a_dst2, b2):
    src = np.concatenate([edge_index[0], np.arange(N_NODES, dtype=np.int64)])
    dst = np.concatenate([edge_index[1], np.arange(N_NODES, dtype=np.int64)])

    core_of = dst // S
    # per-core, per-local-node real degrees
    deg = np.zeros((NC, S), np.int64)
    np.add.at(deg, (core_of, dst % S), 1)

    # degree-rank relabeling per core: rank r <-> local node order[k][r]
    order = np.argsort(-deg, axis=1, kind="stable")     # [NC, S] local idx
    rank_of = np.empty_like(order)
    for k in range(NC):
        rank_of[k, order[k]] = np.arange(S)
    deg_sorted = np.take_along_axis(deg, order, axis=1)
    target_deg = deg_sorted.max(axis=0)                 # [S]

    meta = _pack_structure(target_deg)
    T = len(meta)

    # permuted global numbering: pnode(k, r) = k*S + r  <->  global node
    # global g (in core k=g//S, local l=g%S) -> p = k*S + rank_of[k, l]
    g2p = (core_of_all := np.arange(N_NODES) // S) * S + \
        rank_of[core_of_all, np.arange(N_NODES) % S]
    p2g = np.empty(N_NODES, np.int64)
    p2g[g2p] = np.arange(N_NODES)

    # bucket edges: per (core, rank) list of permuted-src ids
    src_p = g2p[src]
    dst_core = core_of
    dst_rank = rank_of[dst_core, dst % S]
    # sort edges by (core, rank) for contiguous fill
    eorder = np.lexsort((src_p, dst_rank, dst_core))
    src_p_s = src_p[eorder]
    dst_core_s = dst_core[eorder]
    dst_rank_s = dst_rank[eorder]
    # per (core, rank) segment starts
    seg_key = dst_core_s * S + dst_rank_s
    seg_starts = np.searchsorted(seg_key, np.arange(NC * S))

    srcs = np.zeros((NC, T, P), np.int32)
    dlocs = np.zeros((NC, T, P), np.float32)
    for t, (d0, nd, blk, segs) in enumerate(meta):
        lo = d0 - blk * P
        trash = (lo + nd) % P
        dlocs[:, t, :] = trash
        pos = 0
        for r, tcnt in segs:
            slot = r - blk * P
            for k in range(NC):
                s0 = seg_starts[k * S + r]
                dk = int(deg[k, order[k, r]])
                srcs[k, t, pos:pos + dk] = src_p_s[s0:s0 + dk]
                dlocs[k, t, pos:pos + dk] = slot
            pos += tcnt
    dlocT = np.ascontiguousarray(dlocs)                     # [NC, T, P]
    dlocs_pt = np.ascontiguousarray(np.swapaxes(dlocs, 1, 2))  # [NC, P, T]
    srcs_pt = np.ascontiguousarray(np.swapaxes(srcs, 1, 2))    # [NC, P, T]

    # weights (shared across cores)
    bf = ml_dtypes.bfloat16
    W1bf = W1.astype(bf)                                   # [128, 1024]
    W2bf = np.ascontiguousarray(np.swapaxes(W2.astype(bf).reshape(2, P, H * C2), 0, 1))
    wad1 = np.zeros((D_IN, 2 * H), np.float32)
    wad2 = np.zeros((C1, 2 * H), np.float32)
    for h in range(H):
        wad1[:, h] = W1[:, h * C1:(h + 1) * C1] @ a_src1[h]
        wad1[:, H + h] = W1[:, h * C1:(h + 1) * C1] @ a_dst1[h]
        wad2[:, h] = W2[:, h * C2:(h + 1) * C2] @ a_src2[h]
        wad2[:, H + h] = W2[:, h * C2:(h + 1) * C2] @ a_dst2[h]
    e0m = np.zeros((P, P), bf)
    e0m[0, :] = 1.0
    bm1 = np.zeros((P, C1), bf)
    bm1[0, :] = b1.astype(bf)
    bm2 = np.zeros((P, C2), bf)
    bm2[0, :] = b2.astype(bf)

    # per-core transposed inputs, permuted row order
    in_maps = []
    for k in range(NC):
        rows = p2g[k * S:(k + 1) * S]                      # global ids in rank order
        xk = x[rows]                                       # [S, 128]
        in_maps.append({
            "xT": np.ascontiguousarray(xk.T).astype(np.float32),
            "srcs": srcs_pt[k],
            "dlocs": dlocs_pt[k].astype(bf),
            "dlocT": dlocT[k].astype(bf),
            "W1": W1bf, "W2": W2bf,
            "wad1": wad1,
            "wad2": np.ascontiguousarray(np.swapaxes(wad2.reshape(2, P, 2 * H), 0, 1)),
            "e0m": e0m, "bm1": bm1, "bm2": bm2,
        })
    return meta, in_maps, p2g


# --------------------------------------------------------------- device build

def _edge_phase(nc, tc, pools, meta, tbl_full, DBb, C, adst_res, bmat, e0,
                iota_row, iota_col, src_all, dloc_all, dlocT, out_dram):
    dt = _dt
    sb, sm, dtg, ps, po_p = pools
    T = len(meta)
    HCW = H * C // 2          # f32 cols holding the bf16 h block
    ngrp = (T + G8 - 1) // G8
    for g in range(ngrp):
        t_lo, t_hi = g * G8, min((g + 1) * G8, T)
        nt = t_hi - t_lo
        dTg = dtg.tile([P, G8, P], dt.bfloat16, tag="dTg")
        nc.sync.dma_start(dTg[:, :nt, :],
                          dlocT[t_lo:t_hi, :].partition_broadcast(P))
        for t in range(t_lo, t_hi):
            j = t - t_lo
            d0, nd, blk, _segs = meta[t]
            lo = d0 - blk * P
            G = sb.tile([P, DBb], dt.float32, tag="G")
            nc.gpsimd.indirect_dma_start(
                out=G[:], out_offset=None, in_=tbl_full[:],
                in_offset=bass.IndirectOffsetOnAxis(
                    ap=src_all[:, t:t + 1], axis=0))
            Gh = G[:, 0:HCW].bitcast(dt.bfloat16)
            asrc = G[:, HCW:DBb]
            selT = sb.tile([P, P], dt.bfloat16, tag="selT")
            nc.vector.tensor_tensor(
                out=selT[:], in0=dTg[:, j, :], in1=iota_col[:],
                op=mybir.AluOpType.is_equal)
            at = ps.tile([P, 3 * H], dt.float32, tag="attn", space="PSUM")
            ae = at[:, 0:H]
            nc.tensor.matmul(ae, selT[:], adst_res[:, blk, :],
                             start=True, stop=True)
            ev = sm.tile([P, H], dt.float32, tag="ev")
            nc.vector.tensor_add(ev[:], asrc, ae)
            lr = sm.tile([P, H], dt.float32, tag="lr")
            nc.vector.scalar_tensor_tensor(
                out=lr[:], in0=ev[:], scalar=NEG, in1=ev[:],
                op0=mybir.AluOpType.mult, op1=mybir.AluOpType.max)
            p_bf = sm.tile([P, H], dt.bfloat16, tag="p")
            nc.scalar.activation(p_bf[:], lr[:],
                                 mybir.ActivationFunctionType.Exp)
            hi = min(lo + nd + 1, P)
            sel = sb.tile([P, P], dt.bfloat16, tag="sel")
            nc.vector.tensor_tensor(
                out=sel[:, 0:hi],
                in0=dloc_all[:, t:t + 1].to_broadcast([P, hi]),
                in1=iota_row[:, 0:hi], op=mybir.AluOpType.is_equal)
            dn = at[:, H:2 * H]
            nc.tensor.matmul(dn[:hi, :], sel[:, 0:hi], p_bf[:], start=True,
                             stop=True)
            dnc = sm.tile([P, H], dt.float32, tag="dnc")
            nc.vector.tensor_scalar_max(dnc[:hi, :], dn[:hi, :], 1e-30)
            rec = sm.tile([P, H], dt.bfloat16, tag="rec")
            with nc.allow_low_precision(reason="softmax denom reciprocal"):
                nc.vector.reciprocal(rec[:hi, :], dnc[:hi, :])
            re_ps = at[:, 2 * H:3 * H]
            nc.tensor.matmul(re_ps, selT[:hi, :], rec[:hi, :], start=True,
                             stop=True)
            alpha = sm.tile([P, H], dt.float32, tag="al")
            nc.vector.scalar_tensor_tensor(
                out=alpha[:], in0=re_ps, scalar=0.25, in1=p_bf[:],
                op0=mybir.AluOpType.mult, op1=mybir.AluOpType.mult)
            M = sb.tile([P, H * C], dt.bfloat16, tag="M")
            for h in range(H):
                colsl = slice(h * C, (h + 1) * C)
                if h == 0:
                    nc.scalar.activation(
                        M[:, colsl], Gh[:, colsl],
                        mybir.ActivationFunctionType.Copy,
                        scale=alpha[:, h:h + 1])
                else:
                    nc.vector.tensor_scalar_mul(
                        M[:, colsl], Gh[:, colsl], alpha[:, h:h + 1])
            po = po_p.tile([P, C], dt.float32, tag="po", space="PSUM")
            nc.tensor.matmul(po[:nd, :], e0[:, lo:lo + nd], bmat[:],
                             start=True, stop=False)
            for h in range(H):
                nc.tensor.matmul(po[:nd, :], sel[:, lo:lo + nd],
                                 M[:, h * C:(h + 1) * C],
                                 start=False, stop=(h == H - 1))
            ot = sb.tile([P, C], dt.float32, tag="ot")
            nc.scalar.activation(ot[:nd, :], po[:nd, :],
                                 mybir.ActivationFunctionType.Relu)
            if t % 2 == 0:
                nc.sync.dma_start(out_dram[d0:d0 + nd, :], ot[:nd, :])
            else:
                nc.scalar.dma_start(out_dram[d0:d0 + nd, :], ot[:nd, :])


def build_kernel(meta):
    dt = _dt
    T = len(meta)
    nc = bacc.Bacc(None, target_bir_lowering=False)

    xT = nc.dram_tensor("xT", [D_IN, S], dt.float32, kind="ExternalInput")
    srcs = nc.dram_tensor("srcs", [P, T], dt.int32, kind="ExternalInput")
    dlocs = nc.dram_tensor("dlocs", [P, T], dt.bfloat16, kind="ExternalInput")
    dlocT = nc.dram_tensor("dlocT", [T, P], dt.bfloat16, kind="ExternalInput")
    W1 = nc.dram_tensor("W1", [D_IN, H * C1], dt.bfloat16, kind="ExternalInput")
    W2 = nc.dram_tensor("W2", [P, 2, H * C2], dt.bfloat16, kind="ExternalInput")
    wad1 = nc.dram_tensor("wad1", [D_IN, 2 * H], dt.float32, kind="ExternalInput")
    wad2 = nc.dram_tensor("wad2", [P, 2, 2 * H], dt.float32, kind="ExternalInput")
    e0m = nc.dram_tensor("e0m", [P, P], dt.bfloat16, kind="ExternalInput")
    bm1 = nc.dram_tensor("bm1", [P, C1], dt.bfloat16, kind="ExternalInput")
    bm2 = nc.dram_tensor("bm2", [P, C2], dt.bfloat16, kind="ExternalInput")
    h1_out = nc.dram_tensor("h1_out", [S, C1], dt.float32, kind="ExternalOutput")
    h2_out = nc.dram_tensor("h2_out", [S, C2], dt.float32, kind="ExternalOutput")

    ntile = NB  # 128-row node tiles per shard

    with tile.TileContext(nc) as tc:
        with (
            tc.tile_pool(name="const", bufs=1) as constp,
            tc.tile_pool(name="dram", bufs=1, space="DRAM") as dram,
        ):
            # ------- constants
            iota_row_i = constp.tile([P, P], dt.int32, tag="ir_i")
            nc.gpsimd.iota(iota_row_i[:], pattern=[[1, P]], base=0,
                           channel_multiplier=0)
            iota_row = constp.tile([P, P], dt.bfloat16, tag="ir_f")
            nc.vector.tensor_copy(iota_row[:], iota_row_i[:])
            iota_col_i = constp.tile([P, P], dt.int32, tag="ic_i")
            nc.gpsimd.iota(iota_col_i[:], pattern=[[0, P]], base=0,
                           channel_multiplier=1)
            iota_col = constp.tile([P, P], dt.bfloat16, tag="ic_f")
            nc.vector.tensor_copy(iota_col[:], iota_col_i[:])
            e0 = constp.tile([P, P], dt.bfloat16, tag="e0")
            nc.sync.dma_start(e0[:], e0m[:])
            bmat1 = constp.tile([P, C1], dt.bfloat16, tag="bm1")
            nc.sync.dma_start(bmat1[:], bm1[:])
            bmat2 = constp.tile([P, C2], dt.bfloat16, tag="bm2")
            nc.sync.dma_start(bmat2[:], bm2[:])
            src_all = constp.tile([P, T], dt.int32, tag="srca")
            nc.sync.dma_start(src_all[:], srcs[:])
            dloc_all = constp.tile([P, T], dt.bfloat16, tag="dla")
            nc.sync.dma_start(dloc_all[:], dlocs[:])
            w1sb = constp.tile([D_IN, H * C1], dt.bfloat16, tag="w1")
            nc.sync.dma_start(w1sb[:], W1[:])
            w2sb = constp.tile([P, 2, H * C2], dt.bfloat16, tag="w2")
            nc.sync.dma_start(w2sb[:], W2[:])
            wad1sb = constp.tile([D_IN, 2 * H], dt.float32, tag="wa1")
            nc.sync.dma_start(wad1sb[:], wad1[:])
            wad2sb = constp.tile([P, 2, 2 * H], dt.float32, tag="wa2")
            nc.sync.dma_start(wad2sb[:], wad2[:])
            xT_f = constp.tile([D_IN, S], dt.float32, tag="xtf")
            nc.sync.dma_start(xT_f[:], xT[:])
            xT_bf = constp.tile([D_IN, S], dt.bfloat16, tag="xtb")
            nc.vector.tensor_copy(xT_bf[:], xT_f[:])
            adst1_res = constp.tile([P, NB, H], dt.bfloat16, tag="ad1")
            nc.gpsimd.memset(adst1_res[:], 0.0)
            adst2_res = constp.tile([P, NB, H], dt.bfloat16, tag="ad2")
            nc.gpsimd.memset(adst2_res[:], 0.0)

            # ------- DRAM staging
            tbl1_loc = dram.tile([S, DB1], dt.float32, tag="t1l")
            tbl1_full = dram.tile([N_NODES, DB1], dt.float32, tag="t1f")
            tbl2_loc = dram.tile([S, DB2], dt.float32, tag="t2l")
            tbl2_full = dram.tile([N_NODES, DB2], dt.float32, tag="t2f")
            h1_loc = dram.tile([S, C1], dt.float32, tag="h1l")

            # ------- phase 1: L1 projection of own shard
            with tc.tile_pool(name="prj", bufs=2) as prj, \
                 tc.tile_pool(name="prp", bufs=2, space="PSUM") as prp:
                for i in range(ntile):
                    n0 = i * P
                    nt = min(P, S - n0)
                    ph = prp.tile([P, H * C1], dt.float32, tag="ph",
                                  space="PSUM")
                    nc.tensor.matmul(ph[:nt, 0:512], xT_bf[:, n0:n0 + nt],
                                     w1sb[:, 0:512], start=True, stop=True)
                    nc.tensor.matmul(ph[:nt, 512:1024], xT_bf[:, n0:n0 + nt],
                                     w1sb[:, 512:1024], start=True, stop=True)
                    pss = prp.tile([P, 2 * H], dt.float32, tag="ps",
                                  space="PSUM")
                    nc.tensor.matmul(pss[:nt, :], xT_f[:, n0:n0 + nt],
                                     wad1sb[:], start=True, stop=True)
                    tb = prj.tile([P, DB1], dt.float32, tag="tb")
                    tb_h = tb[:, 0:H * C1 // 2].bitcast(dt.bfloat16)
                    nc.scalar.activation(tb_h[:nt, 0:512], ph[:nt, 0:512],
                                         mybir.ActivationFunctionType.Copy)
                    nc.scalar.activation(tb_h[:nt, 512:1024], ph[:nt, 512:1024],
                                         mybir.ActivationFunctionType.Copy)
                    tb_a = tb[:, H * C1 // 2:DB1]
                    nc.vector.tensor_copy(tb_a[:nt, :], pss[:nt, 0:H])
                    nc.vector.tensor_copy(adst1_res[:nt, i, :], pss[:nt, H:2 * H])
                    nc.sync.dma_start(tbl1_loc[n0:n0 + nt, :], tb[:nt, :])

            # ------- phase 2: AllGather table 1
            nc.gpsimd.collective_compute(
                "AllGather", mybir.AluOpType.bypass,
                ins=[tbl1_loc.opt()], outs=[tbl1_full.opt()],
                replica_groups=[list(range(NC))])

            # ------- phase 3: L1 edge phase
            with tc.tile_pool(name="sb", bufs=6) as sb, \
                 tc.tile_pool(name="dtg", bufs=2) as dtg, \
                 tc.tile_pool(name="sm", bufs=10) as sm, \
                 tc.tile_pool(name="ps", bufs=4, space="PSUM") as ps, \
                 tc.tile_pool(name="po_p", bufs=4, space="PSUM") as po_p:
                _edge_phase(nc, tc, (sb, sm, dtg, ps, po_p), meta,
                            tbl1_full, DB1, C1, adst1_res, bmat1, e0,
                            iota_row, iota_col, src_all, dloc_all, dlocT,
                            h1_loc)

            # ------- phase 4: L2 projection (needs own-shard h1 only)
            with tc.tile_pool(name="prj2", bufs=2) as prj2, \
                 tc.tile_pool(name="prp2", bufs=2, space="PSUM") as prp2, \
                 tc.tile_pool(name="h1t", bufs=1) as h1tp:
                ident = constp.tile([P, P], dt.float32, tag="ident")
                from concourse.masks import make_identity
                make_identity(nc, ident[:])
                h1T_f = h1tp.tile([P, 2, S], dt.float32, tag="h1tf")
                h1T_b = h1tp.tile([P, 2, S], dt.bfloat16, tag="h1tb")
                for i in range(ntile):
                    n0 = i * P
                    nt = min(P, S - n0)
                    hrow = prj2.tile([P, C1], dt.float32, tag="hrow")
                    nc.sync.dma_start(hrow[:nt, :], h1_loc[n0:n0 + nt, :])
                    for c in range(2):
                        tp = prp2.tile([P, P], dt.float32, tag="tp",
                                       space="PSUM")
                        nc.tensor.transpose(
                            tp[:, :nt], hrow[:nt, c * P:(c + 1) * P],
                            ident[:nt, :nt])
                        nc.vector.tensor_copy(h1T_f[:, c, n0:n0 + nt],
                                              tp[:, :nt])
                        nc.vector.tensor_copy(h1T_b[:, c, n0:n0 + nt],
                                              tp[:, :nt])
                for i in range(ntile):
                    n0 = i * P
                    nt = min(P, S - n0)
                    ph = prp2.tile([P, H * C2], dt.float32, tag="ph2",
                                   space="PSUM")
                    for c in range(2):
                        nc.tensor.matmul(ph[:nt, :], h1T_b[:, c, n0:n0 + nt],
                                         w2sb[:, c, :],
                                         start=(c == 0), stop=(c == 1))
                    pss = prp2.tile([P, 2 * H], dt.float32, tag="ps2",
                                   space="PSUM")
                    for c in range(2):
                        nc.tensor.matmul(pss[:nt, :], h1T_f[:, c, n0:n0 + nt],
                                         wad2sb[:, c, :],
                                         start=(c == 0), stop=(c == 1))
                    tb = prj2.tile([P, DB2], dt.float32, tag="tb2")
                    tb_h = tb[:, 0:H * C2 // 2].bitcast(dt.bfloat16)
                    nc.scalar.activation(tb_h[:nt, :], ph[:nt, :],
                                         mybir.ActivationFunctionType.Copy)
                    tb_a = tb[:, H * C2 // 2:DB2]
                    nc.vector.tensor_copy(tb_a[:nt, :], pss[:nt, 0:H])
                    nc.vector.tensor_copy(adst2_res[:nt, i, :], pss[:nt, H:2 * H])
                    nc.sync.dma_start(tbl2_loc[n0:n0 + nt, :], tb[:nt, :])

            # ------- phase 5: AllGather table 2
            nc.gpsimd.collective_compute(
                "AllGather", mybir.AluOpType.bypass,
                ins=[tbl2_loc.opt()], outs=[tbl2_full.opt()],
                replica_groups=[list(range(NC))])

            # ------- phase 6: L2 edge phase
            with tc.tile_pool(name="sb2", bufs=6) as sb, \
                 tc.tile_pool(name="dtg2", bufs=2) as dtg, \
                 tc.tile_pool(name="sm2", bufs=10) as sm, \
                 tc.tile_pool(name="ps2", bufs=4, space="PSUM") as ps, \
                 tc.tile_pool(name="po_p2", bufs=4, space="PSUM") as po_p:
                _edge_phase(nc, tc, (sb, sm, dtg, ps, po_p), meta,
                            tbl2_full, DB2, C2, adst2_res, bmat2, e0,
                            iota_row, iota_col, src_all, dloc_all, dlocT,
                            h2_out)

            # ------- final: copy h1 shard to output
            with tc.tile_pool(name="fin", bufs=2) as fin:
                for i in range(ntile):
                    n0 = i * P
                    nt = min(P, S - n0)
                    ft = fin.tile([P, C1], dt.float32, tag="ft")
                    nc.sync.dma_start(ft[:nt, :], h1_loc[n0:n0 + nt, :])
                    nc.sync.dma_start(h1_out[n0:n0 + nt, :], ft[:nt, :])

    nc.compile()
    return nc


_CACHE = {}


def kernel(x, edge_index, W1, a_src1, a_dst1, b1, W2, a_src2, a_dst2, b2):
    x = np.asarray(x, np.float32)
    edge_index = np.asarray(edge_index, np.int64)
    args = tuple(np.asarray(a, np.float32) for a in
                 (W1, a_src1, a_dst1, b1, W2, a_src2, a_dst2, b2))
    meta, in_maps, p2g = _host_prepare(x, edge_index, *args)
    key = len(meta)
    if key not in _CACHE:
        _CACHE[key] = build_kernel(meta)
    nc = _CACHE[key]
    res = run_bass_kernel_spmd(nc, in_maps, core_ids=list(range(NC)))
    h1p = np.concatenate([r["h1_out"] for r in res.results], axis=0)
    h2p = np.concatenate([r["h2_out"] for r in res.results], axis=0)
    # unpermute: row i of (h1p) is permuted node i; p2g[i] = global id
    h1 = np.empty_like(h1p)
    h2 = np.empty_like(h2p)
    h1[p2g] = h1p
    h2[p2g] = h2p
    return (h1, h2)


# revision 15
# speedup vs baseline: 1.0941x; 1.0941x over previous
"""Two-layer GAT (GATConv x2, 4 heads, head-mean) on 8 TRN2 NeuronCores.

Strategy (graph/data parallel):
  - Nodes sharded contiguously across 8 cores (1250 each). Within each
    shard, nodes are relabeled by degree rank so that the tile structure
    (baked into the single SPMD NEFF) is identical across cores; per-node
    degree is padded to the cross-core max at each rank (dummy edges go to
    a per-tile trash slot and do not affect results).
  - Per layer: sharded projection on TensorE (h via bf16 matmul, attention
    logits asrc/adst via f32 matmul against host-folded W@a vectors), then
    AllGather of the packed node table [h(bf16) | asrc(f32)], then the
    edge phase: per 128-edge tile, indirect-gather source rows, tile-local
    segment softmax via selection-matrix matmuls, head-accumulated PSUM
    aggregation seeded with the bias, relu, contiguous output write.
  - Edge tiles are packed per 128-node destination blocks (block-relative
    slots) so alpha_dst stays SBUF-resident; softmax path is f32, message
    path bf16.
"""
import sys

sys.path.insert(0, "/opt/trn_rl_repo")

import numpy as np
import ml_dtypes

import concourse.bass as bass
import concourse.mybir as mybir
import concourse.tile as tile
from concourse import bacc
from concourse.bass_utils import run_bass_kernel_spmd

P = 128
N_NODES = 10000
N_EDGES = 160000
NC = 8
S = N_NODES // NC          # 1250 nodes per core
NB = (S + P - 1) // P      # 10 dst blocks per core
H = 4
C1, C2 = 256, 128
D_IN = 128
DB1 = H * C1 // 2 + H      # 516 f32 cols: bf16 h | f32 asrc
DB2 = H * C2 // 2 + H      # 260 f32 cols
NEG = 0.2
G8 = 16                    # dlocT replicate batch (tiles per DMA)

_dt = mybir.dt


# ----------------------------------------------------------------- host prep

def _pack_structure(target_deg):
    """Tile packing from the (core-independent) per-rank degree sequence.

    Each tile: <=128 edge slots, all edges of a rank-node kept together,
    tiles never span 128-rank block boundaries. Returns per-tile
    (d0, nd, blk, seg list of (rank, count)).
    """
    tiles = []
    cur, cur_cnt = [], 0
    for r in range(S):
        cnt = int(target_deg[r])
        assert cnt <= P, "degree > 128 unsupported"
        blk_changed = cur and (cur[0][0] // P != r // P)
        if cur_cnt + cnt > P or blk_changed:
            tiles.append(cur)
            cur, cur_cnt = [], 0
        cur.append((r, cnt))
        cur_cnt += cnt
    if cur:
        tiles.append(cur)
    meta = []
    for segs in tiles:
        d0 = segs[0][0]
        nd = segs[-1][0] - d0 + 1
        blk = d0 // P
        assert segs[-1][0] // P == blk and nd <= P
        meta.append((d0, nd, blk, segs))
    return meta


def _host_prepare(x, edge_index, W1, a_src1, a_dst1, b1, W2, a_src2, a_dst2, b2):
    src = np.concatenate([edge_index[0], np.arange(N_NODES, dtype=np.int64)])
    dst = np.concatenate([edge_index[1], np.arange(N_NODES, dtype=np.int64)])

    core_of = dst // S
    # per-core, per-local-node real degrees
    deg = np.zeros((NC, S), np.int64)
    np.add.at(deg, (core_of, dst % S), 1)

    # degree-rank relabeling per core: rank r <-> local node order[k][r]
    order = np.argsort(-deg, axis=1, kind="stable")     # [NC, S] local idx
    rank_of = np.empty_like(order)
    for k in range(NC):
        rank_of[k, order[k]] = np.arange(S)
    deg_sorted = np.take_along_axis(deg, order, axis=1)
    target_deg = deg_sorted.max(axis=0)                 # [S]

    meta = _pack_structure(target_deg)
    T = len(meta)

    # permuted global numbering: pnode(k, r) = k*S + r  <->  global node
    # global g (in core k=g//S, local l=g%S) -> p = k*S + rank_of[k, l]
    g2p = (core_of_all := np.arange(N_NODES) // S) * S + \
        rank_of[core_of_all, np.arange(N_NODES) % S]
    p2g = np.empty(N_NODES, np.int64)
    p2g[g2p] = np.arange(N_NODES)

    # bucket edges: per (core, rank) list of permuted-src ids
    src_p = g2p[src]
    dst_core = core_of
    dst_rank = rank_of[dst_core, dst % S]
    # sort edges by (core, rank) for contiguous fill
    eorder = np.lexsort((src_p, dst_rank, dst_core))
    src_p_s = src_p[eorder]
    dst_core_s = dst_core[eorder]
    dst_rank_s = dst_rank[eorder]
    # per (core, rank) segment starts
    seg_key = dst_core_s * S + dst_rank_s
    seg_starts = np.searchsorted(seg_key, np.arange(NC * S))

    srcs = np.zeros((NC, T, P), np.int32)
    dlocs = np.zeros((NC, T, P), np.float32)
    for t, (d0, nd, blk, segs) in enumerate(meta):
        lo = d0 - blk * P
        trash = (lo + nd) % P
        dlocs[:, t, :] = trash
        pos = 0
        for r, tcnt in segs:
            slot = r - blk * P
            for k in range(NC):
                s0 = seg_starts[k * S + r]
                dk = int(deg[k, order[k, r]])
                srcs[k, t, pos:pos + dk] = src_p_s[s0:s0 + dk]
                dlocs[k, t, pos:pos + dk] = slot
            pos += tcnt
    dlocT = np.ascontiguousarray(dlocs)                     # [NC, T, P]
    dlocs_pt = np.ascontiguousarray(np.swapaxes(dlocs, 1, 2))  # [NC, P, T]
    srcs_pt = np.ascontiguousarray(np.swapaxes(srcs, 1, 2))    # [NC, P, T]

    # weights (shared across cores)
    bf = ml_dtypes.bfloat16
    W1bf = W1.astype(bf)                                   # [128, 1024]
    W2bf = np.ascontiguousarray(np.swapaxes(W2.astype(bf).reshape(2, P, H * C2), 0, 1))
    wad1 = np.zeros((D_IN, 2 * H), np.float32)
    wad2 = np.zeros((C1, 2 * H), np.float32)
    for h in range(H):
        wad1[:, h] = W1[:, h * C1:(h + 1) * C1] @ a_src1[h]
        wad1[:, H + h] = W1[:, h * C1:(h + 1) * C1] @ a_dst1[h]
        wad2[:, h] = W2[:, h * C2:(h + 1) * C2] @ a_src2[h]
        wad2[:, H + h] = W2[:, h * C2:(h + 1) * C2] @ a_dst2[h]
    e0m = np.zeros((P, P), bf)
    e0m[0, :] = 1.0
    bm1 = np.zeros((P, C1), bf)
    bm1[0, :] = b1.astype(bf)
    bm2 = np.zeros((P, C2), bf)
    bm2[0, :] = b2.astype(bf)

    # per-core transposed inputs, permuted row order
    in_maps = []
    for k in range(NC):
        rows = p2g[k * S:(k + 1) * S]                      # global ids in rank order
        xk = x[rows]                                       # [S, 128]
        in_maps.append({
            "xT": np.ascontiguousarray(xk.T).astype(np.float32),
            "srcs": srcs_pt[k],
            "dlocs": dlocs_pt[k].astype(bf),
            "dlocT": dlocT[k].astype(bf),
            "W1": W1bf, "W2": W2bf,
            "wad1": wad1,
            "wad2": np.ascontiguousarray(np.swapaxes(wad2.reshape(2, P, 2 * H), 0, 1)),
            "e0m": e0m, "bm1": bm1, "bm2": bm2,
        })
    return meta, in_maps, p2g


# --------------------------------------------------------------- device build

def _edge_phase(nc, tc, pools, meta, tbl_full, DBb, C, adst_res, bmat, e0,
                iota_row, iota_col, src_all, dloc_all, dlocT, out_dram):
    dt = _dt
    sb, sm, dtg, ps, po_p = pools
    T = len(meta)
    HCW = H * C // 2          # f32 cols holding the bf16 h block
    ngrp = (T + G8 - 1) // G8
    for g in range(ngrp):
        t_lo, t_hi = g * G8, min((g + 1) * G8, T)
        nt = t_hi - t_lo
        dTg = dtg.tile([P, G8, P], dt.bfloat16, tag="dTg")
        nc.sync.dma_start(dTg[:, :nt, :],
                          dlocT[t_lo:t_hi, :].partition_broadcast(P))
        for t in range(t_lo, t_hi):
            j = t - t_lo
            d0, nd, blk, _segs = meta[t]
            lo = d0 - blk * P
            G = sb.tile([P, DBb], dt.float32, tag="G")
            nc.gpsimd.indirect_dma_start(
                out=G[:], out_offset=None, in_=tbl_full[:],
                in_offset=bass.IndirectOffsetOnAxis(
                    ap=src_all[:, t:t + 1], axis=0))
            Gh = G[:, 0:HCW].bitcast(dt.bfloat16)
            asrc = G[:, HCW:DBb]
            selT = sb.tile([P, P], dt.bfloat16, tag="selT")
            nc.vector.tensor_tensor(
                out=selT[:], in0=dTg[:, j, :], in1=iota_col[:],
                op=mybir.AluOpType.is_equal)
            at = ps.tile([P, 3 * H], dt.float32, tag="attn", space="PSUM")
            ae = at[:, 0:H]
            nc.tensor.matmul(ae, selT[:], adst_res[:, blk, :],
                             start=True, stop=True)
            ev = sm.tile([P, H], dt.float32, tag="ev")
            nc.vector.tensor_add(ev[:], asrc, ae)
            lr = sm.tile([P, H], dt.float32, tag="lr")
            nc.vector.scalar_tensor_tensor(
                out=lr[:], in0=ev[:], scalar=NEG, in1=ev[:],
                op0=mybir.AluOpType.mult, op1=mybir.AluOpType.max)
            p_bf = sm.tile([P, H], dt.bfloat16, tag="p")
            nc.scalar.activation(p_bf[:], lr[:],
                                 mybir.ActivationFunctionType.Exp)
            hi = min(lo + nd + 1, P)
            sel = sb.tile([P, P], dt.bfloat16, tag="sel")
            nc.vector.tensor_tensor(
                out=sel[:, 0:hi],
                in0=dloc_all[:, t:t + 1].to_broadcast([P, hi]),
                in1=iota_row[:, 0:hi], op=mybir.AluOpType.is_equal)
            dn = at[:, H:2 * H]
            nc.tensor.matmul(dn[:hi, :], sel[:, 0:hi], p_bf[:], start=True,
                             stop=True)
            dnc = sm.tile([P, H], dt.float32, tag="dnc")
            nc.vector.tensor_scalar_max(dnc[:hi, :], dn[:hi, :], 1e-30)
            rec = sm.tile([P, H], dt.bfloat16, tag="rec")
            with nc.allow_low_precision(reason="softmax denom reciprocal"):
                nc.vector.reciprocal(rec[:hi, :], dnc[:hi, :])
            re_ps = at[:, 2 * H:3 * H]
            nc.tensor.matmul(re_ps, selT[:hi, :], rec[:hi, :], start=True,
                             stop=True)
            alpha = sm.tile([P, H], dt.float32, tag="al")
            nc.vector.scalar_tensor_tensor(
                out=alpha[:], in0=re_ps, scalar=0.25, in1=p_bf[:],
                op0=mybir.AluOpType.mult, op1=mybir.AluOpType.mult)
            M = sb.tile([P, H * C], dt.bfloat16, tag="M")
            for h in range(H):
                colsl = slice(h * C, (h + 1) * C)
                if h == 0:
                    nc.scalar.activation(
                        M[:, colsl], Gh[:, colsl],
                        mybir.ActivationFunctionType.Copy,
                        scale=alpha[:, h:h + 1])
                else:
                    nc.vector.tensor_scalar_mul(
                        M[:, colsl], Gh[:, colsl], alpha[:, h:h + 1])
            po = po_p.tile([P, C], dt.float32, tag="po", space="PSUM")
            nc.tensor.matmul(po[:nd, :], e0[:, lo:lo + nd], bmat[:],
                             start=True, stop=False)
            for h in range(H):
                nc.tensor.matmul(po[:nd, :], sel[:, lo:lo + nd],
                                 M[:, h * C:(h + 1) * C],
                                 start=False, stop=(h == H - 1))
            ot = sb.tile([P, C], dt.float32, tag="ot")
            nc.scalar.activation(ot[:nd, :], po[:nd, :],
                                 mybir.ActivationFunctionType.Relu)
            if t % 2 == 0:
                nc.sync.dma_start(out_dram[d0:d0 + nd, :], ot[:nd, :])
            else:
                nc.scalar.dma_start(out_dram[d0:d0 + nd, :], ot[:nd, :])


def build_kernel(meta):
    dt = _dt
    T = len(meta)
    nc = bacc.Bacc(None, target_bir_lowering=False)

    xT = nc.dram_tensor("xT", [D_IN, S], dt.float32, kind="ExternalInput")
    srcs = nc.dram_tensor("srcs", [P, T], dt.int32, kind="ExternalInput")
    dlocs = nc.dram_tensor("dlocs", [P, T], dt.bfloat16, kind="ExternalInput")
    dlocT = nc.dram_tensor("dlocT", [T, P], dt.bfloat16, kind="ExternalInput")
    W1 = nc.dram_tensor("W1", [D_IN, H * C1], dt.bfloat16, kind="ExternalInput")
    W2 = nc.dram_tensor("W2", [P, 2, H * C2], dt.bfloat16, kind="ExternalInput")
    wad1 = nc.dram_tensor("wad1", [D_IN, 2 * H], dt.float32, kind="ExternalInput")
    wad2 = nc.dram_tensor("wad2", [P, 2, 2 * H], dt.float32, kind="ExternalInput")
    e0m = nc.dram_tensor("e0m", [P, P], dt.bfloat16, kind="ExternalInput")
    bm1 = nc.dram_tensor("bm1", [P, C1], dt.bfloat16, kind="ExternalInput")
    bm2 = nc.dram_tensor("bm2", [P, C2], dt.bfloat16, kind="ExternalInput")
    h1_out = nc.dram_tensor("h1_out", [S, C1], dt.float32, kind="ExternalOutput")
    h2_out = nc.dram_tensor("h2_out", [S, C2], dt.float32, kind="ExternalOutput")

    ntile = NB  # 128-row node tiles per shard

    with tile.TileContext(nc) as tc:
        with (
            tc.tile_pool(name="const", bufs=1) as constp,
            tc.tile_pool(name="dram", bufs=1, space="DRAM") as dram,
        ):
            # ------- constants
            iota_row_i = constp.tile([P, P], dt.int32, tag="ir_i")
            nc.gpsimd.iota(iota_row_i[:], pattern=[[1, P]], base=0,
                           channel_multiplier=0)
            iota_row = constp.tile([P, P], dt.bfloat16, tag="ir_f")
            nc.vector.tensor_copy(iota_row[:], iota_row_i[:])
            iota_col_i = constp.tile([P, P], dt.int32, tag="ic_i")
            nc.gpsimd.iota(iota_col_i[:], pattern=[[0, P]], base=0,
                           channel_multiplier=1)
            iota_col = constp.tile([P, P], dt.bfloat16, tag="ic_f")
            nc.vector.tensor_copy(iota_col[:], iota_col_i[:])
            e0 = constp.tile([P, P], dt.bfloat16, tag="e0")
            nc.sync.dma_start(e0[:], e0m[:])
            bmat1 = constp.tile([P, C1], dt.bfloat16, tag="bm1")
            nc.sync.dma_start(bmat1[:], bm1[:])
            bmat2 = constp.tile([P, C2], dt.bfloat16, tag="bm2")
            nc.sync.dma_start(bmat2[:], bm2[:])
            src_all = constp.tile([P, T], dt.int32, tag="srca")
            nc.sync.dma_start(src_all[:], srcs[:])
            dloc_all = constp.tile([P, T], dt.bfloat16, tag="dla")
            nc.sync.dma_start(dloc_all[:], dlocs[:])
            w1sb = constp.tile([D_IN, H * C1], dt.bfloat16, tag="w1")
            nc.sync.dma_start(w1sb[:], W1[:])
            w2sb = constp.tile([P, 2, H * C2], dt.bfloat16, tag="w2")
            nc.sync.dma_start(w2sb[:], W2[:])
            wad1sb = constp.tile([D_IN, 2 * H], dt.float32, tag="wa1")
            nc.sync.dma_start(wad1sb[:], wad1[:])
            wad2sb = constp.tile([P, 2, 2 * H], dt.float32, tag="wa2")
            nc.sync.dma_start(wad2sb[:], wad2[:])
            xT_f = constp.tile([D_IN, S], dt.float32, tag="xtf")
            nc.sync.dma_start(xT_f[:], xT[:])
            xT_bf = constp.tile([D_IN, S], dt.bfloat16, tag="xtb")
            nc.vector.tensor_copy(xT_bf[:], xT_f[:])
            adst1_res = constp.tile([P, NB, H], dt.bfloat16, tag="ad1")
            nc.gpsimd.memset(adst1_res[:], 0.0)
            adst2_res = constp.tile([P, NB, H], dt.bfloat16, tag="ad2")
            nc.gpsimd.memset(adst2_res[:], 0.0)

            # ------- DRAM staging
            tbl1_loc = dram.tile([S, DB1], dt.float32, tag="t1l")
            tbl1_full = dram.tile([N_NODES, DB1], dt.float32, tag="t1f", addr_space="Shared")
            tbl2_loc = dram.tile([S, DB2], dt.float32, tag="t2l")
            tbl2_full = dram.tile([N_NODES, DB2], dt.float32, tag="t2f", addr_space="Shared")
            h1_loc = dram.tile([S, C1], dt.float32, tag="h1l")

            # ------- phase 1: L1 projection of own shard
            with tc.tile_pool(name="prj", bufs=2) as prj, \
                 tc.tile_pool(name="prp", bufs=2, space="PSUM") as prp:
                for i in range(ntile):
                    n0 = i * P
                    nt = min(P, S - n0)
                    ph = prp.tile([P, H * C1], dt.float32, tag="ph",
                                  space="PSUM")
                    nc.tensor.matmul(ph[:nt, 0:512], xT_bf[:, n0:n0 + nt],
                                     w1sb[:, 0:512], start=True, stop=True)
                    nc.tensor.matmul(ph[:nt, 512:1024], xT_bf[:, n0:n0 + nt],
                                     w1sb[:, 512:1024], start=True, stop=True)
                    pss = prp.tile([P, 2 * H], dt.float32, tag="ps",
                                  space="PSUM")
                    nc.tensor.matmul(pss[:nt, :], xT_f[:, n0:n0 + nt],
                                     wad1sb[:], start=True, stop=True)
                    tb = prj.tile([P, DB1], dt.float32, tag="tb")
                    tb_h = tb[:, 0:H * C1 // 2].bitcast(dt.bfloat16)
                    nc.scalar.activation(tb_h[:nt, 0:512], ph[:nt, 0:512],
                                         mybir.ActivationFunctionType.Copy)
                    nc.scalar.activation(tb_h[:nt, 512:1024], ph[:nt, 512:1024],
                                         mybir.ActivationFunctionType.Copy)
                    tb_a = tb[:, H * C1 // 2:DB1]
                    nc.vector.tensor_copy(tb_a[:nt, :], pss[:nt, 0:H])
                    nc.vector.tensor_copy(adst1_res[:nt, i, :], pss[:nt, H:2 * H])
                    nc.sync.dma_start(tbl1_loc[n0:n0 + nt, :], tb[:nt, :])

            # ------- phase 2: AllGather table 1
            nc.gpsimd.collective_compute(
                "AllGather", mybir.AluOpType.bypass,
                ins=[tbl1_loc.opt()], outs=[tbl1_full.opt()],
                replica_groups=[list(range(NC))])

            # ------- phase 3: L1 edge phase
            with tc.tile_pool(name="sb", bufs=6) as sb, \
                 tc.tile_pool(name="dtg", bufs=2) as dtg, \
                 tc.tile_pool(name="sm", bufs=10) as sm, \
                 tc.tile_pool(name="ps", bufs=4, space="PSUM") as ps, \
                 tc.tile_pool(name="po_p", bufs=4, space="PSUM") as po_p:
                _edge_phase(nc, tc, (sb, sm, dtg, ps, po_p), meta,
                            tbl1_full, DB1, C1, adst1_res, bmat1, e0,
                            iota_row, iota_col, src_all, dloc_all, dlocT,
                            h1_loc)

            # ------- phase 4: L2 projection (needs own-shard h1 only)
            with tc.tile_pool(name="prj2", bufs=2) as prj2, \
                 tc.tile_pool(name="prp2", bufs=2, space="PSUM") as prp2, \
                 tc.tile_pool(name="h1t", bufs=1) as h1tp:
                ident = constp.tile([P, P], dt.float32, tag="ident")
                from concourse.masks import make_identity
                make_identity(nc, ident[:])
                h1T_f = h1tp.tile([P, 2, S], dt.float32, tag="h1tf")
                h1T_b = h1tp.tile([P, 2, S], dt.bfloat16, tag="h1tb")
                for i in range(ntile):
                    n0 = i * P
                    nt = min(P, S - n0)
                    hrow = prj2.tile([P, C1], dt.float32, tag="hrow")
                    nc.sync.dma_start(hrow[:nt, :], h1_loc[n0:n0 + nt, :])
                    for c in range(2):
                        tp = prp2.tile([P, P], dt.float32, tag="tp",
                                       space="PSUM")
                        nc.tensor.transpose(
                            tp[:, :nt], hrow[:nt, c * P:(c + 1) * P],
                            ident[:nt, :nt])
                        nc.vector.tensor_copy(h1T_f[:, c, n0:n0 + nt],
                                              tp[:, :nt])
                        nc.vector.tensor_copy(h1T_b[:, c, n0:n0 + nt],
                                              tp[:, :nt])
                for i in range(ntile):
                    n0 = i * P
                    nt = min(P, S - n0)
                    ph = prp2.tile([P, H * C2], dt.float32, tag="ph2",
                                   space="PSUM")
                    for c in range(2):
                        nc.tensor.matmul(ph[:nt, :], h1T_b[:, c, n0:n0 + nt],
                                         w2sb[:, c, :],
                                         start=(c == 0), stop=(c == 1))
                    pss = prp2.tile([P, 2 * H], dt.float32, tag="ps2",
                                   space="PSUM")
                    for c in range(2):
                        nc.tensor.matmul(pss[:nt, :], h1T_f[:, c, n0:n0 + nt],
                                         wad2sb[:, c, :],
                                         start=(c == 0), stop=(c == 1))
                    tb = prj2.tile([P, DB2], dt.float32, tag="tb2")
                    tb_h = tb[:, 0:H * C2 // 2].bitcast(dt.bfloat16)
                    nc.scalar.activation(tb_h[:nt, :], ph[:nt, :],
                                         mybir.ActivationFunctionType.Copy)
                    tb_a = tb[:, H * C2 // 2:DB2]
                    nc.vector.tensor_copy(tb_a[:nt, :], pss[:nt, 0:H])
                    nc.vector.tensor_copy(adst2_res[:nt, i, :], pss[:nt, H:2 * H])
                    nc.sync.dma_start(tbl2_loc[n0:n0 + nt, :], tb[:nt, :])

            # ------- phase 5: AllGather table 2
            nc.gpsimd.collective_compute(
                "AllGather", mybir.AluOpType.bypass,
                ins=[tbl2_loc.opt()], outs=[tbl2_full.opt()],
                replica_groups=[list(range(NC))])

            # ------- phase 6: L2 edge phase
            with tc.tile_pool(name="sb2", bufs=6) as sb, \
                 tc.tile_pool(name="dtg2", bufs=2) as dtg, \
                 tc.tile_pool(name="sm2", bufs=10) as sm, \
                 tc.tile_pool(name="ps2", bufs=4, space="PSUM") as ps, \
                 tc.tile_pool(name="po_p2", bufs=4, space="PSUM") as po_p:
                _edge_phase(nc, tc, (sb, sm, dtg, ps, po_p), meta,
                            tbl2_full, DB2, C2, adst2_res, bmat2, e0,
                            iota_row, iota_col, src_all, dloc_all, dlocT,
                            h2_out)

            # ------- final: copy h1 shard to output
            with tc.tile_pool(name="fin", bufs=2) as fin:
                for i in range(ntile):
                    n0 = i * P
                    nt = min(P, S - n0)
                    ft = fin.tile([P, C1], dt.float32, tag="ft")
                    nc.sync.dma_start(ft[:nt, :], h1_loc[n0:n0 + nt, :])
                    nc.sync.dma_start(h1_out[n0:n0 + nt, :], ft[:nt, :])

    nc.compile()
    return nc


_CACHE = {}


def kernel(x, edge_index, W1, a_src1, a_dst1, b1, W2, a_src2, a_dst2, b2):
    x = np.asarray(x, np.float32)
    edge_index = np.asarray(edge_index, np.int64)
    args = tuple(np.asarray(a, np.float32) for a in
                 (W1, a_src1, a_dst1, b1, W2, a_src2, a_dst2, b2))
    meta, in_maps, p2g = _host_prepare(x, edge_index, *args)
    key = len(meta)
    if key not in _CACHE:
        _CACHE[key] = build_kernel(meta)
    nc = _CACHE[key]
    res = None
    last_err = None
    for _attempt in range(4):
        try:
            res = run_bass_kernel_spmd(nc, in_maps, core_ids=list(range(NC)))
            break
        except Exception as e:  # transient device wedge: retry
            last_err = e
            import time as _time
            _time.sleep(2.0)
    if res is None:
        raise last_err
    h1p = np.concatenate([r["h1_out"] for r in res.results], axis=0)
    h2p = np.concatenate([r["h2_out"] for r in res.results], axis=0)
    # unpermute: row i of (h1p) is permuted node i; p2g[i] = global id
    h1 = np.empty_like(h1p)
    h2 = np.empty_like(h2p)
    h1[p2g] = h1p
    h2[p2g] = h2p
    return (h1, h2)


# revision 16
# speedup vs baseline: 1.1098x; 1.0143x over previous
"""Two-layer GAT (GATConv x2, 4 heads, head-mean) on 8 TRN2 NeuronCores.

Strategy (graph/data parallel):
  - Nodes sharded contiguously across 8 cores (1250 each). Within each
    shard, nodes are relabeled by degree rank so that the tile structure
    (baked into the single SPMD NEFF) is identical across cores; per-node
    degree is padded to the cross-core max at each rank (dummy edges go to
    a per-tile trash slot and do not affect results).
  - Per layer: sharded projection on TensorE (h via bf16 matmul, attention
    logits asrc/adst via f32 matmul against host-folded W@a vectors), then
    AllGather of the packed node table [h(bf16) | asrc(f32)], then the
    edge phase: per 128-edge tile, indirect-gather source rows, tile-local
    segment softmax via selection-matrix matmuls, head-accumulated PSUM
    aggregation seeded with the bias, relu, contiguous output write.
  - Edge tiles are packed per 128-node destination blocks (block-relative
    slots) so alpha_dst stays SBUF-resident; softmax path is f32, message
    path bf16.
"""
import sys

sys.path.insert(0, "/opt/trn_rl_repo")

import numpy as np
import ml_dtypes

import concourse.bass as bass
import concourse.mybir as mybir
import concourse.tile as tile
from concourse import bacc
from concourse.bass_utils import run_bass_kernel_spmd

P = 128
N_NODES = 10000
N_EDGES = 160000
NC = 8
S = N_NODES // NC          # 1250 nodes per core
NB = (S + P - 1) // P      # 10 dst blocks per core
H = 4
C1, C2 = 256, 128
D_IN = 128
DB1 = H * C1 // 2 + H      # 516 f32 cols: bf16 h | f32 asrc
DB2 = H * C2 // 2 + H      # 260 f32 cols
NEG = 0.2
G8 = 32                    # dlocT replicate batch (tiles per DMA)

_dt = mybir.dt


# ----------------------------------------------------------------- host prep

def _pack_structure(target_deg):
    """Tile packing from the (core-independent) per-rank degree sequence.

    Each tile: <=128 edge slots, all edges of a rank-node kept together,
    tiles never span 128-rank block boundaries. Returns per-tile
    (d0, nd, blk, seg list of (rank, count)).
    """
    tiles = []
    cur, cur_cnt = [], 0
    for r in range(S):
        cnt = int(target_deg[r])
        assert cnt <= P, "degree > 128 unsupported"
        blk_changed = cur and (cur[0][0] // P != r // P)
        if cur_cnt + cnt > P or blk_changed:
            tiles.append(cur)
            cur, cur_cnt = [], 0
        cur.append((r, cnt))
        cur_cnt += cnt
    if cur:
        tiles.append(cur)
    meta = []
    for segs in tiles:
        d0 = segs[0][0]
        nd = segs[-1][0] - d0 + 1
        blk = d0 // P
        assert segs[-1][0] // P == blk and nd <= P
        meta.append((d0, nd, blk, segs))
    return meta


def _host_prepare(x, edge_index, W1, a_src1, a_dst1, b1, W2, a_src2, a_dst2, b2):
    src = np.concatenate([edge_index[0], np.arange(N_NODES, dtype=np.int64)])
    dst = np.concatenate([edge_index[1], np.arange(N_NODES, dtype=np.int64)])

    core_of = dst // S
    # per-core, per-local-node real degrees
    deg = np.zeros((NC, S), np.int64)
    np.add.at(deg, (core_of, dst % S), 1)

    # degree-rank relabeling per core: rank r <-> local node order[k][r]
    order = np.argsort(-deg, axis=1, kind="stable")     # [NC, S] local idx
    rank_of = np.empty_like(order)
    for k in range(NC):
        rank_of[k, order[k]] = np.arange(S)
    deg_sorted = np.take_along_axis(deg, order, axis=1)
    target_deg = deg_sorted.max(axis=0)                 # [S]

    meta = _pack_structure(target_deg)
    T = len(meta)

    # permuted global numbering: pnode(k, r) = k*S + r  <->  global node
    # global g (in core k=g//S, local l=g%S) -> p = k*S + rank_of[k, l]
    g2p = (core_of_all := np.arange(N_NODES) // S) * S + \
        rank_of[core_of_all, np.arange(N_NODES) % S]
    p2g = np.empty(N_NODES, np.int64)
    p2g[g2p] = np.arange(N_NODES)

    # bucket edges: per (core, rank) list of permuted-src ids
    src_p = g2p[src]
    dst_core = core_of
    dst_rank = rank_of[dst_core, dst % S]
    # sort edges by (core, rank) for contiguous fill
    eorder = np.lexsort((src_p, dst_rank, dst_core))
    src_p_s = src_p[eorder]
    dst_core_s = dst_core[eorder]
    dst_rank_s = dst_rank[eorder]
    # per (core, rank) segment starts
    seg_key = dst_core_s * S + dst_rank_s
    seg_starts = np.searchsorted(seg_key, np.arange(NC * S))

    srcs = np.zeros((NC, T, P), np.int32)
    dlocs = np.zeros((NC, T, P), np.float32)
    for t, (d0, nd, blk, segs) in enumerate(meta):
        lo = d0 - blk * P
        trash = (lo + nd) % P
        dlocs[:, t, :] = trash
        pos = 0
        for r, tcnt in segs:
            slot = r - blk * P
            for k in range(NC):
                s0 = seg_starts[k * S + r]
                dk = int(deg[k, order[k, r]])
                srcs[k, t, pos:pos + dk] = src_p_s[s0:s0 + dk]
                dlocs[k, t, pos:pos + dk] = slot
            pos += tcnt
    dlocT = np.ascontiguousarray(dlocs)                     # [NC, T, P]
    dlocs_pt = np.ascontiguousarray(np.swapaxes(dlocs, 1, 2))  # [NC, P, T]
    srcs_pt = np.ascontiguousarray(np.swapaxes(srcs, 1, 2))    # [NC, P, T]

    # weights (shared across cores)
    bf = ml_dtypes.bfloat16
    W1bf = W1.astype(bf)                                   # [128, 1024]
    W2bf = np.ascontiguousarray(np.swapaxes(W2.astype(bf).reshape(2, P, H * C2), 0, 1))
    wad1 = np.zeros((D_IN, 2 * H), np.float32)
    wad2 = np.zeros((C1, 2 * H), np.float32)
    for h in range(H):
        wad1[:, h] = W1[:, h * C1:(h + 1) * C1] @ a_src1[h]
        wad1[:, H + h] = W1[:, h * C1:(h + 1) * C1] @ a_dst1[h]
        wad2[:, h] = W2[:, h * C2:(h + 1) * C2] @ a_src2[h]
        wad2[:, H + h] = W2[:, h * C2:(h + 1) * C2] @ a_dst2[h]
    e0m = np.zeros((P, P), bf)
    e0m[0, :] = 1.0
    bm1 = np.zeros((P, C1), bf)
    bm1[0, :] = b1.astype(bf)
    bm2 = np.zeros((P, C2), bf)
    bm2[0, :] = b2.astype(bf)

    # per-core transposed inputs, permuted row order
    in_maps = []
    for k in range(NC):
        rows = p2g[k * S:(k + 1) * S]                      # global ids in rank order
        xk = x[rows]                                       # [S, 128]
        in_maps.append({
            "xT": np.ascontiguousarray(xk.T).astype(np.float32),
            "srcs": srcs_pt[k],
            "dlocs": dlocs_pt[k].astype(bf),
            "dlocT": dlocT[k].astype(bf),
            "W1": W1bf, "W2": W2bf,
            "wad1": wad1,
            "wad2": np.ascontiguousarray(np.swapaxes(wad2.reshape(2, P, 2 * H), 0, 1)),
            "e0m": e0m, "bm1": bm1, "bm2": bm2,
        })
    return meta, in_maps, p2g


# --------------------------------------------------------------- device build

def _edge_phase(nc, tc, pools, meta, tbl_full, DBb, C, adst_res, bmat, e0,
                iota_row, iota_col, src_all, dloc_all, dlocT, out_dram):
    dt = _dt
    sb, sm, dtg, ps, po_p = pools
    T = len(meta)
    HCW = H * C // 2          # f32 cols holding the bf16 h block
    ngrp = (T + G8 - 1) // G8
    for g in range(ngrp):
        t_lo, t_hi = g * G8, min((g + 1) * G8, T)
        nt = t_hi - t_lo
        dTg = dtg.tile([P, G8, P], dt.bfloat16, tag="dTg")
        nc.sync.dma_start(dTg[:, :nt, :],
                          dlocT[t_lo:t_hi, :].partition_broadcast(P))
        for t in range(t_lo, t_hi):
            j = t - t_lo
            d0, nd, blk, _segs = meta[t]
            lo = d0 - blk * P
            G = sb.tile([P, DBb], dt.float32, tag="G")
            nc.gpsimd.indirect_dma_start(
                out=G[:], out_offset=None, in_=tbl_full[:],
                in_offset=bass.IndirectOffsetOnAxis(
                    ap=src_all[:, t:t + 1], axis=0))
            Gh = G[:, 0:HCW].bitcast(dt.bfloat16)
            asrc = G[:, HCW:DBb]
            selT = sb.tile([P, P], dt.bfloat16, tag="selT")
            nc.vector.tensor_tensor(
                out=selT[:], in0=dTg[:, j, :], in1=iota_col[:],
                op=mybir.AluOpType.is_equal)
            at = ps.tile([P, 3 * H], dt.float32, tag="attn", space="PSUM")
            ae = at[:, 0:H]
            nc.tensor.matmul(ae, selT[:], adst_res[:, blk, :],
                             start=True, stop=True)
            ev = sm.tile([P, H], dt.float32, tag="ev")
            nc.vector.tensor_add(ev[:], asrc, ae)
            lr = sm.tile([P, H], dt.float32, tag="lr")
            nc.vector.scalar_tensor_tensor(
                out=lr[:], in0=ev[:], scalar=NEG, in1=ev[:],
                op0=mybir.AluOpType.mult, op1=mybir.AluOpType.max)
            p_bf = sm.tile([P, H], dt.bfloat16, tag="p")
            nc.scalar.activation(p_bf[:], lr[:],
                                 mybir.ActivationFunctionType.Exp)
            hi = min(lo + nd + 1, P)
            sel = sb.tile([P, P], dt.bfloat16, tag="sel")
            nc.vector.tensor_tensor(
                out=sel[:, 0:hi],
                in0=dloc_all[:, t:t + 1].to_broadcast([P, hi]),
                in1=iota_row[:, 0:hi], op=mybir.AluOpType.is_equal)
            dn = at[:, H:2 * H]
            nc.tensor.matmul(dn[:hi, :], sel[:, 0:hi], p_bf[:], start=True,
                             stop=True)
            dnc = sm.tile([P, H], dt.float32, tag="dnc")
            nc.vector.tensor_scalar_max(dnc[:hi, :], dn[:hi, :], 1e-30)
            rec = sm.tile([P, H], dt.bfloat16, tag="rec")
            with nc.allow_low_precision(reason="softmax denom reciprocal"):
                nc.vector.reciprocal(rec[:hi, :], dnc[:hi, :])
            re_ps = at[:, 2 * H:3 * H]
            nc.tensor.matmul(re_ps, selT[:hi, :], rec[:hi, :], start=True,
                             stop=True)
            alpha = sm.tile([P, H], dt.float32, tag="al")
            nc.vector.scalar_tensor_tensor(
                out=alpha[:], in0=re_ps, scalar=0.25, in1=p_bf[:],
                op0=mybir.AluOpType.mult, op1=mybir.AluOpType.mult)
            M = sb.tile([P, H * C], dt.bfloat16, tag="M")
            for h in range(H):
                colsl = slice(h * C, (h + 1) * C)
                if h == 0:
                    nc.scalar.activation(
                        M[:, colsl], Gh[:, colsl],
                        mybir.ActivationFunctionType.Copy,
                        scale=alpha[:, h:h + 1])
                else:
                    nc.vector.tensor_scalar_mul(
                        M[:, colsl], Gh[:, colsl], alpha[:, h:h + 1])
            po = po_p.tile([P, C], dt.float32, tag="po", space="PSUM")
            nc.tensor.matmul(po[:nd, :], e0[:, lo:lo + nd], bmat[:],
                             start=True, stop=False)
            for h in range(H):
                nc.tensor.matmul(po[:nd, :], sel[:, lo:lo + nd],
                                 M[:, h * C:(h + 1) * C],
                                 start=False, stop=(h == H - 1))
            ot = sb.tile([P, C], dt.float32, tag="ot")
            nc.scalar.activation(ot[:nd, :], po[:nd, :],
                                 mybir.ActivationFunctionType.Relu)
            if t % 2 == 0:
                nc.sync.dma_start(out_dram[d0:d0 + nd, :], ot[:nd, :])
            else:
                nc.scalar.dma_start(out_dram[d0:d0 + nd, :], ot[:nd, :])


def build_kernel(meta):
    dt = _dt
    T = len(meta)
    nc = bacc.Bacc(None, target_bir_lowering=False)

    xT = nc.dram_tensor("xT", [D_IN, S], dt.float32, kind="ExternalInput")
    srcs = nc.dram_tensor("srcs", [P, T], dt.int32, kind="ExternalInput")
    dlocs = nc.dram_tensor("dlocs", [P, T], dt.bfloat16, kind="ExternalInput")
    dlocT = nc.dram_tensor("dlocT", [T, P], dt.bfloat16, kind="ExternalInput")
    W1 = nc.dram_tensor("W1", [D_IN, H * C1], dt.bfloat16, kind="ExternalInput")
    W2 = nc.dram_tensor("W2", [P, 2, H * C2], dt.bfloat16, kind="ExternalInput")
    wad1 = nc.dram_tensor("wad1", [D_IN, 2 * H], dt.float32, kind="ExternalInput")
    wad2 = nc.dram_tensor("wad2", [P, 2, 2 * H], dt.float32, kind="ExternalInput")
    e0m = nc.dram_tensor("e0m", [P, P], dt.bfloat16, kind="ExternalInput")
    bm1 = nc.dram_tensor("bm1", [P, C1], dt.bfloat16, kind="ExternalInput")
    bm2 = nc.dram_tensor("bm2", [P, C2], dt.bfloat16, kind="ExternalInput")
    h1_out = nc.dram_tensor("h1_out", [S, C1], dt.float32, kind="ExternalOutput")
    h2_out = nc.dram_tensor("h2_out", [S, C2], dt.float32, kind="ExternalOutput")

    ntile = NB  # 128-row node tiles per shard

    with tile.TileContext(nc) as tc:
        with (
            tc.tile_pool(name="const", bufs=1) as constp,
            tc.tile_pool(name="dram", bufs=1, space="DRAM") as dram,
        ):
            # ------- constants
            iota_row_i = constp.tile([P, P], dt.int32, tag="ir_i")
            nc.gpsimd.iota(iota_row_i[:], pattern=[[1, P]], base=0,
                           channel_multiplier=0)
            iota_row = constp.tile([P, P], dt.bfloat16, tag="ir_f")
            nc.vector.tensor_copy(iota_row[:], iota_row_i[:])
            iota_col_i = constp.tile([P, P], dt.int32, tag="ic_i")
            nc.gpsimd.iota(iota_col_i[:], pattern=[[0, P]], base=0,
                           channel_multiplier=1)
            iota_col = constp.tile([P, P], dt.bfloat16, tag="ic_f")
            nc.vector.tensor_copy(iota_col[:], iota_col_i[:])
            e0 = constp.tile([P, P], dt.bfloat16, tag="e0")
            nc.sync.dma_start(e0[:], e0m[:])
            bmat1 = constp.tile([P, C1], dt.bfloat16, tag="bm1")
            nc.sync.dma_start(bmat1[:], bm1[:])
            bmat2 = constp.tile([P, C2], dt.bfloat16, tag="bm2")
            nc.sync.dma_start(bmat2[:], bm2[:])
            src_all = constp.tile([P, T], dt.int32, tag="srca")
            nc.sync.dma_start(src_all[:], srcs[:])
            dloc_all = constp.tile([P, T], dt.bfloat16, tag="dla")
            nc.sync.dma_start(dloc_all[:], dlocs[:])
            w1sb = constp.tile([D_IN, H * C1], dt.bfloat16, tag="w1")
            nc.sync.dma_start(w1sb[:], W1[:])
            w2sb = constp.tile([P, 2, H * C2], dt.bfloat16, tag="w2")
            nc.sync.dma_start(w2sb[:], W2[:])
            wad1sb = constp.tile([D_IN, 2 * H], dt.float32, tag="wa1")
            nc.sync.dma_start(wad1sb[:], wad1[:])
            wad2sb = constp.tile([P, 2, 2 * H], dt.float32, tag="wa2")
            nc.sync.dma_start(wad2sb[:], wad2[:])
            xT_f = constp.tile([D_IN, S], dt.float32, tag="xtf")
            nc.sync.dma_start(xT_f[:], xT[:])
            xT_bf = constp.tile([D_IN, S], dt.bfloat16, tag="xtb")
            nc.vector.tensor_copy(xT_bf[:], xT_f[:])
            adst1_res = constp.tile([P, NB, H], dt.bfloat16, tag="ad1")
            nc.gpsimd.memset(adst1_res[:], 0.0)
            adst2_res = constp.tile([P, NB, H], dt.bfloat16, tag="ad2")
            nc.gpsimd.memset(adst2_res[:], 0.0)

            # ------- DRAM staging
            tbl1_loc = dram.tile([S, DB1], dt.float32, tag="t1l")
            tbl1_full = dram.tile([N_NODES, DB1], dt.float32, tag="t1f", addr_space="Shared")
            tbl2_loc = dram.tile([S, DB2], dt.float32, tag="t2l")
            tbl2_full = dram.tile([N_NODES, DB2], dt.float32, tag="t2f", addr_space="Shared")
            h1_loc = dram.tile([S, C1], dt.float32, tag="h1l")

            # ------- phase 1: L1 projection of own shard
            with tc.tile_pool(name="prj", bufs=2) as prj, \
                 tc.tile_pool(name="prp", bufs=2, space="PSUM") as prp:
                for i in range(ntile):
                    n0 = i * P
                    nt = min(P, S - n0)
                    ph = prp.tile([P, H * C1], dt.float32, tag="ph",
                                  space="PSUM")
                    nc.tensor.matmul(ph[:nt, 0:512], xT_bf[:, n0:n0 + nt],
                                     w1sb[:, 0:512], start=True, stop=True)
                    nc.tensor.matmul(ph[:nt, 512:1024], xT_bf[:, n0:n0 + nt],
                                     w1sb[:, 512:1024], start=True, stop=True)
                    pss = prp.tile([P, 2 * H], dt.float32, tag="ps",
                                  space="PSUM")
                    nc.tensor.matmul(pss[:nt, :], xT_f[:, n0:n0 + nt],
                                     wad1sb[:], start=True, stop=True)
                    tb = prj.tile([P, DB1], dt.float32, tag="tb")
                    tb_h = tb[:, 0:H * C1 // 2].bitcast(dt.bfloat16)
                    nc.scalar.activation(tb_h[:nt, 0:512], ph[:nt, 0:512],
                                         mybir.ActivationFunctionType.Copy)
                    nc.scalar.activation(tb_h[:nt, 512:1024], ph[:nt, 512:1024],
                                         mybir.ActivationFunctionType.Copy)
                    tb_a = tb[:, H * C1 // 2:DB1]
                    nc.vector.tensor_copy(tb_a[:nt, :], pss[:nt, 0:H])
                    nc.vector.tensor_copy(adst1_res[:nt, i, :], pss[:nt, H:2 * H])
                    nc.sync.dma_start(tbl1_loc[n0:n0 + nt, :], tb[:nt, :])

            # ------- phase 2: AllGather table 1
            nc.gpsimd.collective_compute(
                "AllGather", mybir.AluOpType.bypass,
                ins=[tbl1_loc.opt()], outs=[tbl1_full.opt()],
                replica_groups=[list(range(NC))])

            # ------- phase 3: L1 edge phase
            with tc.tile_pool(name="sb", bufs=8) as sb, \
                 tc.tile_pool(name="dtg", bufs=3) as dtg, \
                 tc.tile_pool(name="sm", bufs=12) as sm, \
                 tc.tile_pool(name="ps", bufs=4, space="PSUM") as ps, \
                 tc.tile_pool(name="po_p", bufs=4, space="PSUM") as po_p:
                _edge_phase(nc, tc, (sb, sm, dtg, ps, po_p), meta,
                            tbl1_full, DB1, C1, adst1_res, bmat1, e0,
                            iota_row, iota_col, src_all, dloc_all, dlocT,
                            h1_loc)

            # ------- phase 4: L2 projection (needs own-shard h1 only)
            with tc.tile_pool(name="prj2", bufs=2) as prj2, \
                 tc.tile_pool(name="prp2", bufs=2, space="PSUM") as prp2, \
                 tc.tile_pool(name="h1t", bufs=1) as h1tp:
                ident = constp.tile([P, P], dt.float32, tag="ident")
                from concourse.masks import make_identity
                make_identity(nc, ident[:])
                h1T_f = h1tp.tile([P, 2, S], dt.float32, tag="h1tf")
                h1T_b = h1tp.tile([P, 2, S], dt.bfloat16, tag="h1tb")
                for i in range(ntile):
                    n0 = i * P
                    nt = min(P, S - n0)
                    hrow = prj2.tile([P, C1], dt.float32, tag="hrow")
                    nc.sync.dma_start(hrow[:nt, :], h1_loc[n0:n0 + nt, :])
                    for c in range(2):
                        tp = prp2.tile([P, P], dt.float32, tag="tp",
                                       space="PSUM")
                        nc.tensor.transpose(
                            tp[:, :nt], hrow[:nt, c * P:(c + 1) * P],
                            ident[:nt, :nt])
                        nc.vector.tensor_copy(h1T_f[:, c, n0:n0 + nt],
                                              tp[:, :nt])
                        nc.vector.tensor_copy(h1T_b[:, c, n0:n0 + nt],
                                              tp[:, :nt])
                for i in range(ntile):
                    n0 = i * P
                    nt = min(P, S - n0)
                    ph = prp2.tile([P, H * C2], dt.float32, tag="ph2",
                                   space="PSUM")
                    for c in range(2):
                        nc.tensor.matmul(ph[:nt, :], h1T_b[:, c, n0:n0 + nt],
                                         w2sb[:, c, :],
                                         start=(c == 0), stop=(c == 1))
                    pss = prp2.tile([P, 2 * H], dt.float32, tag="ps2",
                                   space="PSUM")
                    for c in range(2):
                        nc.tensor.matmul(pss[:nt, :], h1T_f[:, c, n0:n0 + nt],
                                         wad2sb[:, c, :],
                                         start=(c == 0), stop=(c == 1))
                    tb = prj2.tile([P, DB2], dt.float32, tag="tb2")
                    tb_h = tb[:, 0:H * C2 // 2].bitcast(dt.bfloat16)
                    nc.scalar.activation(tb_h[:nt, :], ph[:nt, :],
                                         mybir.ActivationFunctionType.Copy)
                    tb_a = tb[:, H * C2 // 2:DB2]
                    nc.vector.tensor_copy(tb_a[:nt, :], pss[:nt, 0:H])
                    nc.vector.tensor_copy(adst2_res[:nt, i, :], pss[:nt, H:2 * H])
                    nc.sync.dma_start(tbl2_loc[n0:n0 + nt, :], tb[:nt, :])

            # ------- phase 5: AllGather table 2
            nc.gpsimd.collective_compute(
                "AllGather", mybir.AluOpType.bypass,
                ins=[tbl2_loc.opt()], outs=[tbl2_full.opt()],
                replica_groups=[list(range(NC))])

            # ------- phase 6: L2 edge phase
            with tc.tile_pool(name="sb2", bufs=8) as sb, \
                 tc.tile_pool(name="dtg2", bufs=3) as dtg, \
                 tc.tile_pool(name="sm2", bufs=12) as sm, \
                 tc.tile_pool(name="ps2", bufs=4, space="PSUM") as ps, \
                 tc.tile_pool(name="po_p2", bufs=4, space="PSUM") as po_p:
                _edge_phase(nc, tc, (sb, sm, dtg, ps, po_p), meta,
                            tbl2_full, DB2, C2, adst2_res, bmat2, e0,
                            iota_row, iota_col, src_all, dloc_all, dlocT,
                            h2_out)

            # ------- final: copy h1 shard to output
            with tc.tile_pool(name="fin", bufs=2) as fin:
                for i in range(ntile):
                    n0 = i * P
                    nt = min(P, S - n0)
                    ft = fin.tile([P, C1], dt.float32, tag="ft")
                    nc.sync.dma_start(ft[:nt, :], h1_loc[n0:n0 + nt, :])
                    nc.sync.dma_start(h1_out[n0:n0 + nt, :], ft[:nt, :])

    nc.compile()
    return nc


_CACHE = {}


def kernel(x, edge_index, W1, a_src1, a_dst1, b1, W2, a_src2, a_dst2, b2):
    x = np.asarray(x, np.float32)
    edge_index = np.asarray(edge_index, np.int64)
    args = tuple(np.asarray(a, np.float32) for a in
                 (W1, a_src1, a_dst1, b1, W2, a_src2, a_dst2, b2))
    meta, in_maps, p2g = _host_prepare(x, edge_index, *args)
    key = len(meta)
    if key not in _CACHE:
        _CACHE[key] = build_kernel(meta)
    nc = _CACHE[key]
    res = None
    last_err = None
    for _attempt in range(4):
        try:
            res = run_bass_kernel_spmd(nc, in_maps, core_ids=list(range(NC)))
            break
        except Exception as e:  # transient device wedge: retry
            last_err = e
            import time as _time
            _time.sleep(2.0)
    if res is None:
        raise last_err
    h1p = np.concatenate([r["h1_out"] for r in res.results], axis=0)
    h2p = np.concatenate([r["h2_out"] for r in res.results], axis=0)
    # unpermute: row i of (h1p) is permuted node i; p2g[i] = global id
    h1 = np.empty_like(h1p)
    h2 = np.empty_like(h2p)
    h1[p2g] = h1p
    h2[p2g] = h2p
    return (h1, h2)


# revision 17
# speedup vs baseline: 1.1248x; 1.0135x over previous
"""Two-layer GAT (GATConv x2, 4 heads, head-mean) on 8 TRN2 NeuronCores.

Strategy (graph/data parallel):
  - Nodes sharded contiguously across 8 cores (1250 each). Within each
    shard, nodes are relabeled by degree rank so that the tile structure
    (baked into the single SPMD NEFF) is identical across cores; per-node
    degree is padded to the cross-core max at each rank (dummy edges go to
    a per-tile trash slot and do not affect results).
  - Per layer: sharded projection on TensorE (h via bf16 matmul, attention
    logits asrc/adst via f32 matmul against host-folded W@a vectors), then
    AllGather of the packed node table [h(bf16) | asrc(f32)], then the
    edge phase: per 128-edge tile, indirect-gather source rows, tile-local
    segment softmax via selection-matrix matmuls, head-accumulated PSUM
    aggregation seeded with the bias, relu, contiguous output write.
  - Edge tiles are packed per 128-node destination blocks (block-relative
    slots) so alpha_dst stays SBUF-resident; softmax path is f32, message
    path bf16.
"""
import sys

sys.path.insert(0, "/opt/trn_rl_repo")

import numpy as np
import ml_dtypes

import concourse.bass as bass
import concourse.mybir as mybir
import concourse.tile as tile
from concourse import bacc
from concourse.bass_utils import run_bass_kernel_spmd

P = 128
N_NODES = 10000
N_EDGES = 160000
NC = 8
S = N_NODES // NC          # 1250 nodes per core
NB = (S + P - 1) // P      # 10 dst blocks per core
H = 4
C1, C2 = 256, 128
D_IN = 128
DB1 = H * C1 // 2 + H      # 516 f32 cols: bf16 h | f32 asrc
DB2 = H * C2 // 2 + H      # 260 f32 cols
NEG = 0.2
G8 = 32                    # dlocT replicate batch (tiles per DMA)

_dt = mybir.dt


# ----------------------------------------------------------------- host prep

def _pack_structure(target_deg):
    """Tile packing from the (core-independent) per-rank degree sequence.

    Each tile: <=128 edge slots, all edges of a rank-node kept together,
    tiles never span 128-rank block boundaries. Returns per-tile
    (d0, nd, blk, seg list of (rank, count)).
    """
    tiles = []
    cur, cur_cnt = [], 0
    for r in range(S):
        cnt = int(target_deg[r])
        assert cnt <= P, "degree > 128 unsupported"
        blk_changed = cur and (cur[0][0] // P != r // P)
        if cur_cnt + cnt > P or blk_changed:
            tiles.append(cur)
            cur, cur_cnt = [], 0
        cur.append((r, cnt))
        cur_cnt += cnt
    if cur:
        tiles.append(cur)
    meta = []
    for segs in tiles:
        d0 = segs[0][0]
        nd = segs[-1][0] - d0 + 1
        blk = d0 // P
        assert segs[-1][0] // P == blk and nd <= P
        meta.append((d0, nd, blk, segs))
    return meta


def _host_prepare(x, edge_index, W1, a_src1, a_dst1, b1, W2, a_src2, a_dst2, b2):
    src = np.concatenate([edge_index[0], np.arange(N_NODES, dtype=np.int64)])
    dst = np.concatenate([edge_index[1], np.arange(N_NODES, dtype=np.int64)])

    core_of = dst // S
    # per-core, per-local-node real degrees
    deg = np.zeros((NC, S), np.int64)
    np.add.at(deg, (core_of, dst % S), 1)

    # degree-rank relabeling per core: rank r <-> local node order[k][r]
    order = np.argsort(-deg, axis=1, kind="stable")     # [NC, S] local idx
    rank_of = np.empty_like(order)
    for k in range(NC):
        rank_of[k, order[k]] = np.arange(S)
    deg_sorted = np.take_along_axis(deg, order, axis=1)
    target_deg = deg_sorted.max(axis=0)                 # [S]

    meta = _pack_structure(target_deg)
    T = len(meta)

    # permuted global numbering: pnode(k, r) = k*S + r  <->  global node
    # global g (in core k=g//S, local l=g%S) -> p = k*S + rank_of[k, l]
    g2p = (core_of_all := np.arange(N_NODES) // S) * S + \
        rank_of[core_of_all, np.arange(N_NODES) % S]
    p2g = np.empty(N_NODES, np.int64)
    p2g[g2p] = np.arange(N_NODES)

    # bucket edges: per (core, rank) list of permuted-src ids
    src_p = g2p[src]
    dst_core = core_of
    dst_rank = rank_of[dst_core, dst % S]
    # sort edges by (core, rank) for contiguous fill
    eorder = np.lexsort((src_p, dst_rank, dst_core))
    src_p_s = src_p[eorder]
    dst_core_s = dst_core[eorder]
    dst_rank_s = dst_rank[eorder]
    # per (core, rank) segment starts
    seg_key = dst_core_s * S + dst_rank_s
    seg_starts = np.searchsorted(seg_key, np.arange(NC * S))

    srcs = np.zeros((NC, T, P), np.int32)
    dlocs = np.zeros((NC, T, P), np.float32)
    for t, (d0, nd, blk, segs) in enumerate(meta):
        lo = d0 - blk * P
        trash = (lo + nd) % P
        dlocs[:, t, :] = trash
        pos = 0
        for r, tcnt in segs:
            slot = r - blk * P
            for k in range(NC):
                s0 = seg_starts[k * S + r]
                dk = int(deg[k, order[k, r]])
                srcs[k, t, pos:pos + dk] = src_p_s[s0:s0 + dk]
                dlocs[k, t, pos:pos + dk] = slot
            pos += tcnt
    dlocT = np.ascontiguousarray(dlocs)                     # [NC, T, P]
    dlocs_pt = np.ascontiguousarray(np.swapaxes(dlocs, 1, 2))  # [NC, P, T]
    srcs_pt = np.ascontiguousarray(np.swapaxes(srcs, 1, 2))    # [NC, P, T]

    # weights (shared across cores)
    bf = ml_dtypes.bfloat16
    W1bf = W1.astype(bf)                                   # [128, 1024]
    W2bf = np.ascontiguousarray(np.swapaxes(W2.astype(bf).reshape(2, P, H * C2), 0, 1))
    wad1 = np.zeros((D_IN, 2 * H), np.float32)
    wad2 = np.zeros((C1, 2 * H), np.float32)
    for h in range(H):
        wad1[:, h] = W1[:, h * C1:(h + 1) * C1] @ a_src1[h]
        wad1[:, H + h] = W1[:, h * C1:(h + 1) * C1] @ a_dst1[h]
        wad2[:, h] = W2[:, h * C2:(h + 1) * C2] @ a_src2[h]
        wad2[:, H + h] = W2[:, h * C2:(h + 1) * C2] @ a_dst2[h]
    e0m = np.zeros((P, P), bf)
    e0m[0, :] = 1.0
    bm1 = np.zeros((P, C1), bf)
    bm1[0, :] = b1.astype(bf)
    bm2 = np.zeros((P, C2), bf)
    bm2[0, :] = b2.astype(bf)

    # per-core transposed inputs, permuted row order
    in_maps = []
    for k in range(NC):
        rows = p2g[k * S:(k + 1) * S]                      # global ids in rank order
        xk = x[rows]                                       # [S, 128]
        in_maps.append({
            "xT": np.ascontiguousarray(xk.T).astype(np.float32),
            "srcs": srcs_pt[k],
            "dlocs": dlocs_pt[k].astype(bf),
            "dlocT": dlocT[k].astype(bf),
            "W1": W1bf, "W2": W2bf,
            "wad1": wad1,
            "wad2": np.ascontiguousarray(np.swapaxes(wad2.reshape(2, P, 2 * H), 0, 1)),
            "e0m": e0m, "bm1": bm1, "bm2": bm2,
        })
    return meta, in_maps, p2g


# --------------------------------------------------------------- device build

def _edge_phase(nc, tc, pools, meta, tbl_full, DBb, C, adst_res, bmat, e0,
                iota_row, iota_col, src_all, dloc_all, dlocT, out_dram):
    dt = _dt
    sb, sm, dtg, ps, po_p = pools
    T = len(meta)
    HCW = H * C // 2          # f32 cols holding the bf16 h block
    ngrp = (T + G8 - 1) // G8
    for g in range(ngrp):
        t_lo, t_hi = g * G8, min((g + 1) * G8, T)
        nt = t_hi - t_lo
        dTg = dtg.tile([P, G8, P], dt.bfloat16, tag="dTg")
        nc.sync.dma_start(dTg[:, :nt, :],
                          dlocT[t_lo:t_hi, :].partition_broadcast(P))
        for t in range(t_lo, t_hi):
            j = t - t_lo
            d0, nd, blk, _segs = meta[t]
            lo = d0 - blk * P
            G = sb.tile([P, DBb], dt.float32, tag="G")
            nc.gpsimd.indirect_dma_start(
                out=G[:], out_offset=None, in_=tbl_full[:],
                in_offset=bass.IndirectOffsetOnAxis(
                    ap=src_all[:, t:t + 1], axis=0))
            Gh = G[:, 0:HCW].bitcast(dt.bfloat16)
            asrc = G[:, HCW:DBb]
            selT = sb.tile([P, P], dt.bfloat16, tag="selT")
            nc.vector.tensor_tensor(
                out=selT[:], in0=dTg[:, j, :], in1=iota_col[:],
                op=mybir.AluOpType.is_equal)
            at = ps.tile([P, 3 * H], dt.float32, tag="attn", space="PSUM")
            ae = at[:, 0:H]
            nc.tensor.matmul(ae, selT[:], adst_res[:, blk, :],
                             start=True, stop=True)
            ev = sm.tile([P, H], dt.float32, tag="ev")
            nc.vector.tensor_add(ev[:], asrc, ae)
            lr = sm.tile([P, H], dt.float32, tag="lr")
            nc.vector.scalar_tensor_tensor(
                out=lr[:], in0=ev[:], scalar=NEG, in1=ev[:],
                op0=mybir.AluOpType.mult, op1=mybir.AluOpType.max)
            p_bf = sm.tile([P, H], dt.bfloat16, tag="p")
            nc.scalar.activation(p_bf[:], lr[:],
                                 mybir.ActivationFunctionType.Exp)
            hi = min(lo + nd + 1, P)
            sel = sb.tile([P, P], dt.bfloat16, tag="sel")
            nc.vector.tensor_tensor(
                out=sel[:, 0:hi],
                in0=dloc_all[:, t:t + 1].to_broadcast([P, hi]),
                in1=iota_row[:, 0:hi], op=mybir.AluOpType.is_equal)
            dn = at[:, H:2 * H]
            nc.tensor.matmul(dn[:hi, :], sel[:, 0:hi], p_bf[:], start=True,
                             stop=True)
            dnc = sm.tile([P, H], dt.float32, tag="dnc")
            nc.vector.tensor_scalar_max(dnc[:hi, :], dn[:hi, :], 1e-30)
            rec = sm.tile([P, H], dt.bfloat16, tag="rec")
            with nc.allow_low_precision(reason="softmax denom reciprocal"):
                nc.vector.reciprocal(rec[:hi, :], dnc[:hi, :])
            re_ps = at[:, 2 * H:3 * H]
            nc.tensor.matmul(re_ps, selT[:hi, :], rec[:hi, :], start=True,
                             stop=True)
            alpha = sm.tile([P, H], dt.float32, tag="al")
            nc.vector.scalar_tensor_tensor(
                out=alpha[:], in0=re_ps, scalar=0.25, in1=p_bf[:],
                op0=mybir.AluOpType.mult, op1=mybir.AluOpType.mult)
            M = sb.tile([P, H * C], dt.bfloat16, tag="M")
            for h in range(H):
                colsl = slice(h * C, (h + 1) * C)
                if h == 0:
                    nc.scalar.activation(
                        M[:, colsl], Gh[:, colsl],
                        mybir.ActivationFunctionType.Copy,
                        scale=alpha[:, h:h + 1])
                else:
                    nc.vector.tensor_scalar_mul(
                        M[:, colsl], Gh[:, colsl], alpha[:, h:h + 1])
            po = po_p.tile([P, C], dt.float32, tag="po", space="PSUM")
            nc.tensor.matmul(po[:nd, :], e0[:, lo:lo + nd], bmat[:],
                             start=True, stop=False)
            for h in range(H):
                nc.tensor.matmul(po[:nd, :], sel[:, lo:lo + nd],
                                 M[:, h * C:(h + 1) * C],
                                 start=False, stop=(h == H - 1))
            ot = sb.tile([P, C], dt.float32, tag="ot")
            nc.scalar.activation(ot[:nd, :], po[:nd, :],
                                 mybir.ActivationFunctionType.Relu)
            if t % 2 == 0:
                nc.sync.dma_start(out_dram[d0:d0 + nd, :], ot[:nd, :])
            else:
                nc.scalar.dma_start(out_dram[d0:d0 + nd, :], ot[:nd, :])


def build_kernel(meta):
    dt = _dt
    T = len(meta)
    nc = bacc.Bacc(None, target_bir_lowering=False)

    xT = nc.dram_tensor("xT", [D_IN, S], dt.float32, kind="ExternalInput")
    srcs = nc.dram_tensor("srcs", [P, T], dt.int32, kind="ExternalInput")
    dlocs = nc.dram_tensor("dlocs", [P, T], dt.bfloat16, kind="ExternalInput")
    dlocT = nc.dram_tensor("dlocT", [T, P], dt.bfloat16, kind="ExternalInput")
    W1 = nc.dram_tensor("W1", [D_IN, H * C1], dt.bfloat16, kind="ExternalInput")
    W2 = nc.dram_tensor("W2", [P, 2, H * C2], dt.bfloat16, kind="ExternalInput")
    wad1 = nc.dram_tensor("wad1", [D_IN, 2 * H], dt.float32, kind="ExternalInput")
    wad2 = nc.dram_tensor("wad2", [P, 2, 2 * H], dt.float32, kind="ExternalInput")
    e0m = nc.dram_tensor("e0m", [P, P], dt.bfloat16, kind="ExternalInput")
    bm1 = nc.dram_tensor("bm1", [P, C1], dt.bfloat16, kind="ExternalInput")
    bm2 = nc.dram_tensor("bm2", [P, C2], dt.bfloat16, kind="ExternalInput")
    h1_out = nc.dram_tensor("h1_out", [S, C1], dt.float32, kind="ExternalOutput")
    h2_out = nc.dram_tensor("h2_out", [S, C2], dt.float32, kind="ExternalOutput")

    ntile = NB  # 128-row node tiles per shard

    with tile.TileContext(nc) as tc:
        with (
            tc.tile_pool(name="const", bufs=1) as constp,
            tc.tile_pool(name="dram", bufs=1, space="DRAM") as dram,
        ):
            # ------- constants
            iota_row_i = constp.tile([P, P], dt.int32, tag="ir_i")
            nc.gpsimd.iota(iota_row_i[:], pattern=[[1, P]], base=0,
                           channel_multiplier=0)
            iota_row = constp.tile([P, P], dt.bfloat16, tag="ir_f")
            nc.vector.tensor_copy(iota_row[:], iota_row_i[:])
            iota_col_i = constp.tile([P, P], dt.int32, tag="ic_i")
            nc.gpsimd.iota(iota_col_i[:], pattern=[[0, P]], base=0,
                           channel_multiplier=1)
            iota_col = constp.tile([P, P], dt.bfloat16, tag="ic_f")
            nc.vector.tensor_copy(iota_col[:], iota_col_i[:])
            e0 = constp.tile([P, P], dt.bfloat16, tag="e0")
            nc.sync.dma_start(e0[:], e0m[:])
            bmat1 = constp.tile([P, C1], dt.bfloat16, tag="bm1")
            nc.sync.dma_start(bmat1[:], bm1[:])
            bmat2 = constp.tile([P, C2], dt.bfloat16, tag="bm2")
            nc.sync.dma_start(bmat2[:], bm2[:])
            src_all = constp.tile([P, T], dt.int32, tag="srca")
            nc.sync.dma_start(src_all[:], srcs[:])
            dloc_all = constp.tile([P, T], dt.bfloat16, tag="dla")
            nc.sync.dma_start(dloc_all[:], dlocs[:])
            w1sb = constp.tile([D_IN, H * C1], dt.bfloat16, tag="w1")
            nc.sync.dma_start(w1sb[:], W1[:])
            w2sb = constp.tile([P, 2, H * C2], dt.bfloat16, tag="w2")
            nc.sync.dma_start(w2sb[:], W2[:])
            wad1sb = constp.tile([D_IN, 2 * H], dt.float32, tag="wa1")
            nc.sync.dma_start(wad1sb[:], wad1[:])
            wad2sb = constp.tile([P, 2, 2 * H], dt.float32, tag="wa2")
            nc.sync.dma_start(wad2sb[:], wad2[:])
            xT_f = constp.tile([D_IN, S], dt.float32, tag="xtf")
            nc.sync.dma_start(xT_f[:], xT[:])
            xT_bf = constp.tile([D_IN, S], dt.bfloat16, tag="xtb")
            nc.vector.tensor_copy(xT_bf[:], xT_f[:])
            adst1_res = constp.tile([P, NB, H], dt.bfloat16, tag="ad1")
            nc.gpsimd.memset(adst1_res[:], 0.0)
            adst2_res = constp.tile([P, NB, H], dt.bfloat16, tag="ad2")
            nc.gpsimd.memset(adst2_res[:], 0.0)

            # ------- DRAM staging
            tbl1_loc = dram.tile([S, DB1], dt.float32, tag="t1l")
            tbl1_full = dram.tile([N_NODES, DB1], dt.float32, tag="t1f", addr_space="Shared")
            tbl2_loc = dram.tile([S, DB2], dt.float32, tag="t2l")
            tbl2_full = dram.tile([N_NODES, DB2], dt.float32, tag="t2f", addr_space="Shared")
            h1_loc = dram.tile([S, C1], dt.float32, tag="h1l")

            # ------- phase 1: L1 projection of own shard
            with tc.tile_pool(name="prj", bufs=2) as prj, \
                 tc.tile_pool(name="prp", bufs=2, space="PSUM") as prp:
                for i in range(ntile):
                    n0 = i * P
                    nt = min(P, S - n0)
                    ph = prp.tile([P, H * C1], dt.float32, tag="ph",
                                  space="PSUM")
                    nc.tensor.matmul(ph[:nt, 0:512], xT_bf[:, n0:n0 + nt],
                                     w1sb[:, 0:512], start=True, stop=True)
                    nc.tensor.matmul(ph[:nt, 512:1024], xT_bf[:, n0:n0 + nt],
                                     w1sb[:, 512:1024], start=True, stop=True)
                    pss = prp.tile([P, 2 * H], dt.float32, tag="ps",
                                  space="PSUM")
                    nc.tensor.matmul(pss[:nt, :], xT_f[:, n0:n0 + nt],
                                     wad1sb[:], start=True, stop=True)
                    tb = prj.tile([P, DB1], dt.float32, tag="tb")
                    tb_h = tb[:, 0:H * C1 // 2].bitcast(dt.bfloat16)
                    nc.scalar.activation(tb_h[:nt, 0:512], ph[:nt, 0:512],
                                         mybir.ActivationFunctionType.Copy)
                    nc.scalar.activation(tb_h[:nt, 512:1024], ph[:nt, 512:1024],
                                         mybir.ActivationFunctionType.Copy)
                    tb_a = tb[:, H * C1 // 2:DB1]
                    nc.vector.tensor_copy(tb_a[:nt, :], pss[:nt, 0:H])
                    nc.vector.tensor_copy(adst1_res[:nt, i, :], pss[:nt, H:2 * H])
                    nc.sync.dma_start(tbl1_loc[n0:n0 + nt, :], tb[:nt, :])

            # ------- phase 2: AllGather table 1
            nc.gpsimd.collective_compute(
                "AllGather", mybir.AluOpType.bypass,
                ins=[tbl1_loc.opt()], outs=[tbl1_full.opt()],
                replica_groups=[list(range(NC))])

            # ------- phase 3: L1 edge phase
            with tc.tile_pool(name="sb", bufs=10) as sb, \
                 tc.tile_pool(name="dtg", bufs=3) as dtg, \
                 tc.tile_pool(name="sm", bufs=14) as sm, \
                 tc.tile_pool(name="ps", bufs=4, space="PSUM") as ps, \
                 tc.tile_pool(name="po_p", bufs=4, space="PSUM") as po_p:
                _edge_phase(nc, tc, (sb, sm, dtg, ps, po_p), meta,
                            tbl1_full, DB1, C1, adst1_res, bmat1, e0,
                            iota_row, iota_col, src_all, dloc_all, dlocT,
                            h1_loc)

            # ------- phase 4: L2 projection (needs own-shard h1 only)
            with tc.tile_pool(name="prj2", bufs=2) as prj2, \
                 tc.tile_pool(name="prp2", bufs=2, space="PSUM") as prp2, \
                 tc.tile_pool(name="h1t", bufs=1) as h1tp:
                ident = constp.tile([P, P], dt.float32, tag="ident")
                from concourse.masks import make_identity
                make_identity(nc, ident[:])
                h1T_f = h1tp.tile([P, 2, S], dt.float32, tag="h1tf")
                h1T_b = h1tp.tile([P, 2, S], dt.bfloat16, tag="h1tb")
                for i in range(ntile):
                    n0 = i * P
                    nt = min(P, S - n0)
                    hrow = prj2.tile([P, C1], dt.float32, tag="hrow")
                    nc.sync.dma_start(hrow[:nt, :], h1_loc[n0:n0 + nt, :])
                    for c in range(2):
                        tp = prp2.tile([P, P], dt.float32, tag="tp",
                                       space="PSUM")
                        nc.tensor.transpose(
                            tp[:, :nt], hrow[:nt, c * P:(c + 1) * P],
                            ident[:nt, :nt])
                        nc.vector.tensor_copy(h1T_f[:, c, n0:n0 + nt],
                                              tp[:, :nt])
                        nc.vector.tensor_copy(h1T_b[:, c, n0:n0 + nt],
                                              tp[:, :nt])
                for i in range(ntile):
                    n0 = i * P
                    nt = min(P, S - n0)
                    ph = prp2.tile([P, H * C2], dt.float32, tag="ph2",
                                   space="PSUM")
                    for c in range(2):
                        nc.tensor.matmul(ph[:nt, :], h1T_b[:, c, n0:n0 + nt],
                                         w2sb[:, c, :],
                                         start=(c == 0), stop=(c == 1))
                    pss = prp2.tile([P, 2 * H], dt.float32, tag="ps2",
                                   space="PSUM")
                    for c in range(2):
                        nc.tensor.matmul(pss[:nt, :], h1T_f[:, c, n0:n0 + nt],
                                         wad2sb[:, c, :],
                                         start=(c == 0), stop=(c == 1))
                    tb = prj2.tile([P, DB2], dt.float32, tag="tb2")
                    tb_h = tb[:, 0:H * C2 // 2].bitcast(dt.bfloat16)
                    nc.scalar.activation(tb_h[:nt, :], ph[:nt, :],
                                         mybir.ActivationFunctionType.Copy)
                    tb_a = tb[:, H * C2 // 2:DB2]
                    nc.vector.tensor_copy(tb_a[:nt, :], pss[:nt, 0:H])
                    nc.vector.tensor_copy(adst2_res[:nt, i, :], pss[:nt, H:2 * H])
                    nc.sync.dma_start(tbl2_loc[n0:n0 + nt, :], tb[:nt, :])

            # ------- phase 5: AllGather table 2
            nc.gpsimd.collective_compute(
                "AllGather", mybir.AluOpType.bypass,
                ins=[tbl2_loc.opt()], outs=[tbl2_full.opt()],
                replica_groups=[list(range(NC))])

            # ------- phase 6: L2 edge phase
            with tc.tile_pool(name="sb2", bufs=10) as sb, \
                 tc.tile_pool(name="dtg2", bufs=3) as dtg, \
                 tc.tile_pool(name="sm2", bufs=14) as sm, \
                 tc.tile_pool(name="ps2", bufs=4, space="PSUM") as ps, \
                 tc.tile_pool(name="po_p2", bufs=4, space="PSUM") as po_p:
                _edge_phase(nc, tc, (sb, sm, dtg, ps, po_p), meta,
                            tbl2_full, DB2, C2, adst2_res, bmat2, e0,
                            iota_row, iota_col, src_all, dloc_all, dlocT,
                            h2_out)

            # ------- final: copy h1 shard to output
            with tc.tile_pool(name="fin", bufs=2) as fin:
                for i in range(ntile):
                    n0 = i * P
                    nt = min(P, S - n0)
                    ft = fin.tile([P, C1], dt.float32, tag="ft")
                    nc.sync.dma_start(ft[:nt, :], h1_loc[n0:n0 + nt, :])
                    nc.sync.dma_start(h1_out[n0:n0 + nt, :], ft[:nt, :])

    nc.compile()
    return nc


_CACHE = {}


def kernel(x, edge_index, W1, a_src1, a_dst1, b1, W2, a_src2, a_dst2, b2):
    x = np.asarray(x, np.float32)
    edge_index = np.asarray(edge_index, np.int64)
    args = tuple(np.asarray(a, np.float32) for a in
                 (W1, a_src1, a_dst1, b1, W2, a_src2, a_dst2, b2))
    meta, in_maps, p2g = _host_prepare(x, edge_index, *args)
    key = len(meta)
    if key not in _CACHE:
        _CACHE[key] = build_kernel(meta)
    nc = _CACHE[key]
    res = None
    last_err = None
    for _attempt in range(4):
        try:
            res = run_bass_kernel_spmd(nc, in_maps, core_ids=list(range(NC)))
            break
        except Exception as e:  # transient device wedge: retry
            last_err = e
            import time as _time
            _time.sleep(2.0)
    if res is None:
        raise last_err
    h1p = np.concatenate([r["h1_out"] for r in res.results], axis=0)
    h2p = np.concatenate([r["h2_out"] for r in res.results], axis=0)
    # unpermute: row i of (h1p) is permuted node i; p2g[i] = global id
    h1 = np.empty_like(h1p)
    h2 = np.empty_like(h2p)
    h1[p2g] = h1p
    h2[p2g] = h2p
    return (h1, h2)
